# revision 19
# baseline (speedup 1.0000x reference)
"""Trainium2 Bass kernel for MultiHeadLatentAttention (MLA), 8-core SPMD.

Sharding: core c = (batch b=c//4, head-group g=c%4 of 4 heads).
Each core computes the full latent down-projections for its batch
(replicated across the 4 cores of that batch), head-sharded
up-projections + RoPE + causal attention, and a partial o-projection
(its heads' rows of Wo). The host sums the 4 partial outputs per batch.

Shapes (fixed): B=2, S=2048, H=2048, L=256, nh=16, hd=128, rd=64.

All matmul operands are bf16 (f32 PSUM accumulation); the host passes
hidden_states pre-transposed ([H, s], features on partitions) so the
device runs no transposes at all.

Device layouts (features on partitions):
  hsT         [128, NKC, s-block] streamed per s-block from DRAM
  kv_dT, q_dT [L=256 -> 2x128, s]
  k_rT        [2x128, s] chunk0 = all heads' rope-lo rows (4x32),
                         chunk1 = all heads' rope-hi rows; rotated in place
  qT_h, kT_h  [128 (64 content + 64 rope), s] per head
  v_all       [128 (s%128), s//128, 4 heads * 128]  (natural v)
  yT_all      [128 (hd), 4 heads, s]

Attention is computed in scores-transposed orientation S^T[k, q] so the
probabilities feed the AV matmul directly (lhsT = v block, rhs = expS).
Softmax skips the max-subtraction (scores are tiny here: |s| < ~2).
Causal structure: k-blocks strictly above the diagonal are skipped, and
diagonal k-blocks compute only the live column range, with a single
[128,128] affine_select for the triangular edge. The denominator is
accumulated per k-block by an all-ones matmul into PSUM (broadcasts it
across partitions); the reciprocal runs on the ACT engine straight from
PSUM and one vector multiply normalizes y.
"""

import os
import sys

sys.path.insert(0, "/opt/trn_rl_repo")

import numpy as np

B = 2
S = 2048
H = 2048
L = 256          # latent dim (2 chunks of 128)
NH = 16          # total heads
HPC = 4          # heads per core
HD = 128         # head dim
RD = 64          # rope / content half-width
ROPE_BASE = 10000.0
SCALE = float(HD) ** -0.5

SB = 512         # s-block for projections / q-blocks in attention
KB = 128         # k-block in attention
NKC = H // 128   # 16 contraction chunks over H
NLC = L // 128   # 2 contraction chunks over L

# Matmul input dtype: "bf16" (fast, ~2e-3 rel err) or "f32r" (~3e-4).
MM_DTYPE = os.environ.get("MLA_MM_DTYPE", "bf16")


def build_nc(s=S, mm_dtype=MM_DTYPE):
    """Build the Bass module for one core. `s` can be shrunk (multiple of 512)
    for simulator testing."""
    from concourse import bacc
    import concourse.bass as bass
    import concourse.mybir as mybir
    import concourse.tile as tile
    from contextlib import ExitStack

    f32 = mybir.dt.float32
    mdt = mybir.dt.bfloat16 if mm_dtype == "bf16" else mybir.dt.float32r

    nsb = s // SB        # s-blocks
    nsc = s // 128       # 128-row s-chunks

    nc = bacc.Bacc(None, target_bir_lowering=False)

    hsT = nc.dram_tensor("hsT", [H, s], mdt, kind="ExternalInput")
    # this core's s-slice of hsT (host-sliced so the program is uniform
    # across cores): down-projections run only on the local quarter and
    # the batch group all-gathers the latents
    hsT_down = nc.dram_tensor("hsT_down", [H, SB], mdt, kind="ExternalInput")
    w_down = nc.dram_tensor("w_down", [H, 512], mdt, kind="ExternalInput")
    w_rk = nc.dram_tensor("w_rk", [H, HPC * RD], mdt, kind="ExternalInput")
    w_qc = nc.dram_tensor("w_qc", [L, HPC * RD], mdt, kind="ExternalInput")
    w_qr = nc.dram_tensor("w_qr", [L, HPC * RD], mdt, kind="ExternalInput")
    w_ku = nc.dram_tensor("w_ku", [L, HPC * RD], mdt, kind="ExternalInput")
    w_vu = nc.dram_tensor("w_vu", [L, HPC * HD], mdt, kind="ExternalInput")
    w_o = nc.dram_tensor("w_o", [HPC * HD, H], mdt, kind="ExternalInput")
    ones_in = nc.dram_tensor("ones_in", [128, 128], mdt, kind="ExternalInput")
    # cos/sin halves replicated across all four 32-partition quadrants so any
    # 32-row operand can pair with a table slice at the SAME base partition
    # (walrus: both-SBUF tensor_tensor inputs must share base partition).
    rope_cc = nc.dram_tensor("rope_cc", [128, s], mdt, kind="ExternalInput")
    rope_ss = nc.dram_tensor("rope_ss", [128, s], mdt, kind="ExternalInput")
    out = nc.dram_tensor("out", [s, H], mdt, kind="ExternalOutput")

    Exp = mybir.ActivationFunctionType.Exp
    Ln = mybir.ActivationFunctionType.Ln
    is_ge = mybir.AluOpType.is_ge

    with ExitStack() as top:
        tc = top.enter_context(tile.TileContext(nc))

        # ---- persistent small pools -------------------------------------
        const_pool = top.enter_context(tc.tile_pool(name="const", bufs=1))
        ones128 = const_pool.tile([128, 128], mdt, tag="ones")
        cc_t = const_pool.tile([128, s], mdt, tag="ropec")
        ss_t = const_pool.tile([128, s], mdt, tag="ropes")

        wsmall = top.enter_context(tc.tile_pool(name="wsmall", bufs=1))
        w_qc_t = wsmall.tile([128, NLC, HPC * RD], mdt, tag="wqc")
        w_qr_t = wsmall.tile([128, NLC, HPC * RD], mdt, tag="wqr")
        w_ku_t = wsmall.tile([128, NLC, HPC * RD], mdt, tag="wku")
        w_vu_t = wsmall.tile([128, NLC, HPC * HD], mdt, tag="wvu")

        # ---- latent / rope-k tensors (live through all C1 phases) -------
        lat_pool = top.enter_context(tc.tile_pool(name="lat", bufs=1))
        kv_dT = lat_pool.tile([128, NLC, s], mdt, tag="kvd")   # [L, s]
        q_dT = lat_pool.tile([128, NLC, s], mdt, tag="qd")     # [L, s]
        k_rT = lat_pool.tile([128, NLC, s], mdt, tag="krt")    # rope k rows

        # ================= PHASE A: down/rope projections ================
        # A-1: kv/q down-projections on this core's s-quarter only, then
        # AllGather the latents across the 4 cores of the batch group
        # while A-2 (head-local rope-k over the full sequence) keeps the
        # PE busy.
        scr_in = nc.dram_tensor("scr_in", [4, 128, SB], mdt, kind="Internal")
        scr_gath = nc.dram_tensor("scr_gath", [4, 4, 128, SB], mdt,
                                  kind="Internal")
        cc_groups = [[0, 1, 2, 3], [4, 5, 6, 7]]

        hsT_r = hsT.rearrange("(ko p) m -> p ko m", p=128)
        hsTd_r = hsT_down.rearrange("(ko p) m -> p ko m", p=128)
        with ExitStack() as pa:
            hstp = pa.enter_context(tc.tile_pool(name="hst", bufs=2))
            hsdp = pa.enter_context(tc.tile_pool(name="hsd", bufs=1))
            wdp = pa.enter_context(tc.tile_pool(name="wdown", bufs=1))
            wrkp = pa.enter_context(tc.tile_pool(name="wrk", bufs=1))
            latp = pa.enter_context(tc.tile_pool(name="latp", bufs=1))
            psa = pa.enter_context(tc.tile_pool(name="psa", bufs=1, space="PSUM"))

            # per-chunk weight DMAs so the first matmuls don't wait for the
            # whole weight tensor
            w_down_t = wdp.tile([128, NKC, 512], mdt, tag="wd")
            w_rk_t = wrkp.tile([128, NKC, HPC * RD], mdt, tag="wr")
            wd_r = w_down.rearrange("(ko p) m -> p ko m", p=128)
            wr_r = w_rk.rearrange("(ko p) m -> p ko m", p=128)

            # ---- A-1: local down-projections + AllGather ---------------
            hsDt = hsdp.tile([128, NKC, SB], mdt, tag="hsD")
            for g in range(4):
                nc.gpsimd.dma_start(hsDt[:, 4 * g:4 * g + 4, :],
                                    hsTd_r[:, 4 * g:4 * g + 4, :])
            for kc in range(NKC):
                nc.sync.dma_start(w_down_t[:, kc, :], wd_r[:, kc, :])
                nc.sync.dma_start(w_rk_t[:, kc, :], wr_r[:, kc, :])

            down_names = ("kv0", "kv1", "q0", "q1")
            pb = {n: psa.tile([128, SB], f32, tag=f"psa_{n}",
                              name=f"psa_{n}") for n in down_names}
            for kc in range(NKC):
                rhs = hsDt[:, kc, :]
                st = kc == 0
                sp = kc == NKC - 1
                for i, n in enumerate(down_names):
                    nc.tensor.matmul(pb[n][:],
                                     w_down_t[:, kc, 128 * i:128 * (i + 1)],
                                     rhs, start=st, stop=sp)
            lat_part = latp.tile([128, 4, SB], mdt, tag="latpart")
            nc.scalar.copy(lat_part[:, 0, :], pb["kv0"][:])
            nc.vector.tensor_copy(lat_part[:, 1, :], pb["kv1"][:])
            nc.scalar.copy(lat_part[:, 2, :], pb["q0"][:])
            nc.vector.tensor_copy(lat_part[:, 3, :], pb["q1"][:])
            for ch in range(4):
                nc.sync.dma_start(scr_in[ch, :, :], lat_part[:, ch, :])
            nc.gpsimd.collective_compute(
                kind="AllGather",
                op=mybir.AluOpType.bypass,
                replica_groups=cc_groups,
                ins=[scr_in.ap()],
                outs=[scr_gath.ap()],
            )
            for rank in range(min(4, nsb)):    # nsb<4 only in sim shrink
                rs = slice(rank * SB, (rank + 1) * SB)
                nc.sync.dma_start(kv_dT[:, 0, rs], scr_gath[rank, 0, :, :])
                nc.sync.dma_start(kv_dT[:, 1, rs], scr_gath[rank, 1, :, :])
                nc.sync.dma_start(q_dT[:, 0, rs], scr_gath[rank, 2, :, :])
                nc.sync.dma_start(q_dT[:, 1, rs], scr_gath[rank, 3, :, :])

            # ---- A-2: rope-k over the full sequence (head-local) -------
            for sb in range(nsb):
                sbs = slice(sb * SB, (sb + 1) * SB)
                hsTt = hstp.tile([128, NKC, SB], mdt, tag="hsT")
                for g in range(4):
                    nc.gpsimd.dma_start(hsTt[:, 4 * g:4 * g + 4, :],
                                        hsT_r[:, 4 * g:4 * g + 4, sbs])
                pr0 = psa.tile([128, SB], f32, tag="psa_kr0",
                               name=f"psa_kr0_{sb}")
                pr1 = psa.tile([128, SB], f32, tag="psa_kr1",
                               name=f"psa_kr1_{sb}")
                for kc in range(NKC):
                    rhs = hsTt[:, kc, :]
                    st = kc == 0
                    sp = kc == NKC - 1
                    nc.tensor.matmul(pr0[:], w_rk_t[:, kc, 0:128],
                                     rhs, start=st, stop=sp)
                    nc.tensor.matmul(pr1[:], w_rk_t[:, kc, 128:256],
                                     rhs, start=st, stop=sp)
                nc.scalar.copy(k_rT[:, 0, sbs], pr0[:])
                nc.vector.tensor_copy(k_rT[:, 1, sbs], pr1[:])

        # late small-weight / rope-table loads: queued behind phase A's DMAs
        nc.sync.dma_start(ones128[:], ones_in[:])
        nc.sync.dma_start(cc_t[:], rope_cc[:])
        nc.sync.dma_start(ss_t[:], rope_ss[:])
        nc.sync.dma_start(
            w_qc_t[:], w_qc.rearrange("(ko p) m -> p ko m", p=128))
        nc.sync.dma_start(
            w_qr_t[:], w_qr.rearrange("(ko p) m -> p ko m", p=128))
        nc.sync.dma_start(
            w_ku_t[:], w_ku.rearrange("(ko p) m -> p ko m", p=128))
        nc.sync.dma_start(
            w_vu_t[:], w_vu.rearrange("(ko p) m -> p ko m", p=128))

        # ---- rotate k_rT in place (RoPE on all 4 heads at once) ---------
        # chunk0 rows = all heads' lo dims, chunk1 = hi dims:
        #   lo' = lo*cos - hi*sin ;  hi' = hi*cos + lo*sin
        with tc.tile_pool(name="rkr", bufs=2) as rkr:
            for sb in range(nsb):
                sbs = slice(sb * SB, (sb + 1) * SB)
                r1 = rkr.tile([128, SB], mdt, tag="r1")
                r2 = rkr.tile([128, SB], mdt, tag="r2")
                r3 = rkr.tile([128, SB], mdt, tag="r3")
                r4 = rkr.tile([128, SB], mdt, tag="r4")
                lo = k_rT[:, 0, sbs]
                hi = k_rT[:, 1, sbs]
                nc.vector.tensor_mul(r1[:], lo, cc_t[:, sbs])
                nc.vector.tensor_mul(r2[:], hi, ss_t[:, sbs])
                nc.vector.tensor_mul(r3[:], hi, cc_t[:, sbs])
                nc.vector.tensor_mul(r4[:], lo, ss_t[:, sbs])
                nc.vector.tensor_sub(k_rT[:, 0, sbs], r1[:], r2[:])
                nc.vector.tensor_add(k_rT[:, 1, sbs], r3[:], r4[:])

        # ================= PHASES B-C per head pair ======================
        vp = top.enter_context(tc.tile_pool(name="vp", bufs=1))
        yp = top.enter_context(tc.tile_pool(name="yp", bufs=1))
        yT_all = yp.tile([128, HPC, s], mdt, tag="yT")

        with ExitStack() as pbc:
            qkp = pbc.enter_context(tc.tile_pool(name="qkp", bufs=2))
            esp = pbc.enter_context(tc.tile_pool(name="esp", bufs=3))
            recp = pbc.enter_context(tc.tile_pool(name="recp", bufs=2))
            rtmp = pbc.enter_context(tc.tile_pool(name="rtmp", bufs=1))
            # one shared [128,SB] psum tag for scores / C1 / v blocks
            psC = pbc.enter_context(
                tc.tile_pool(name="psC", bufs=4, space="PSUM"))
            ps_y = pbc.enter_context(
                tc.tile_pool(name="ps_y", bufs=2, space="PSUM"))
            ps_b = pbc.enter_context(
                tc.tile_pool(name="ps_b", bufs=2, space="PSUM"))

            wop = pbc.enter_context(tc.tile_pool(name="wop", bufs=4))
            outp = pbc.enter_context(tc.tile_pool(name="outp", bufs=4))

            nqb = s // SB
            dpq = SB // KB                      # diagonal blocks per qi

            # o-proj weights preloaded on the gpsimd DMA queue (first
            # consumed once q-block 0 of all heads is done)
            wo_tiles = []
            for ncol in range(H // 512):
                wo_t = wop.tile([128, HPC, 512], mdt, tag="wo",
                                name=f"wo_{ncol}")
                nc.gpsimd.dma_start(
                    wo_t[:],
                    w_o[:, ncol * 512:(ncol + 1) * 512].rearrange(
                        "(ho p) m -> p ho m", p=128))
                wo_tiles.append(wo_t)

            # all 4 heads' q/k tensors and both pairs' v live concurrently
            # (qi-outer loop): C2 for q-block qi only needs kT/qT/v up to
            # s-block qi, and the o-projection for q-block qi-1 runs as PE
            # filler while the next C1's ACT/DVE chains drain.
            qT = {h: qkp.tile([128, s], mdt, tag="qT", bufs=4,
                              name=f"qT_{h}") for h in range(2 * HPC // 2)}
            kT = {h: qkp.tile([128, s], mdt, tag="kT", bufs=4,
                              name=f"kT_{h}") for h in range(2 * HPC // 2)}
            v_pair = {hp: vp.tile([128, nsc, 2 * HD], mdt, tag="vpair",
                                  bufs=2, name=f"v_pair_{hp}")
                      for hp in range(2)}

            def emit_c1_b(hp, sb):
                h0, h1 = 2 * hp, 2 * hp + 1
                sbs = slice(sb * SB, (sb + 1) * SB)
                # q rope FIRST: its DVE chain is the longest pole to the
                # first score matmul of this q-block.
                # psum rows = [h0_lo, h1_lo, h0_hi, h1_hi] (32 each)
                pr = psC.tile([128, SB], f32, tag="blk",
                              name=f"pr_{hp}_{sb}")
                for lc in range(NLC):
                    nc.tensor.matmul(
                        pr[:],
                        w_qr_t[:, lc, hp * 128:(hp + 1) * 128],
                        q_dT[:, lc, sbs],
                        start=(lc == 0), stop=(lc == NLC - 1))
                t1 = rtmp.tile([64, SB], mdt, tag="t1")
                t2 = rtmp.tile([64, SB], mdt, tag="t2")
                t3 = rtmp.tile([64, SB], mdt, tag="t3")
                t4 = rtmp.tile([64, SB], mdt, tag="t4")
                nc.vector.tensor_mul(t1[:], pr[0:64, :], cc_t[0:64, sbs])
                nc.vector.tensor_mul(t2[:], pr[64:128, :], ss_t[64:128, sbs])
                nc.vector.tensor_mul(t3[:], pr[64:128, :], cc_t[64:128, sbs])
                nc.vector.tensor_mul(t4[:], pr[0:64, :], ss_t[0:64, sbs])
                # write rotated rows straight into each head's qT
                nc.vector.tensor_sub(qT[h0][64:96, sbs],
                                     t1[0:32, :], t2[0:32, :])
                nc.vector.tensor_sub(qT[h1][64:96, sbs],
                                     t1[32:64, :], t2[32:64, :])
                nc.vector.tensor_add(qT[h0][96:128, sbs],
                                     t3[0:32, :], t4[0:32, :])
                nc.vector.tensor_add(qT[h1][96:128, sbs],
                                     t3[32:64, :], t4[32:64, :])

                # k content for both heads in one [128, SB] psum.  All
                # evictions go to DVE so ACT stays nearly exp-only (the
                # exp feeds the AV matmul critical path).
                pk = psC.tile([128, SB], f32, tag="blk",
                              name=f"pk_{hp}_{sb}")
                for lc in range(NLC):
                    nc.tensor.matmul(
                        pk[:],
                        w_ku_t[:, lc, hp * 128:(hp + 1) * 128],
                        kv_dT[:, lc, sbs],
                        start=(lc == 0), stop=(lc == NLC - 1))
                nc.vector.tensor_copy(kT[h0][0:64, sbs], pk[0:64, :])
                nc.vector.tensor_copy(kT[h1][0:32, sbs], pk[64:96, :])
                nc.vector.tensor_copy(kT[h1][32:64, sbs], pk[96:128, :])
                # k rope: copy pre-rotated k_rT rows on DVE
                for h in (h0, h1):
                    rb = slice(32 * h, 32 * h + 32)
                    nc.vector.tensor_copy(kT[h][64:96, sbs],
                                          k_rT[rb, 0, sbs])
                    nc.vector.tensor_copy(kT[h][96:128, sbs],
                                          k_rT[rb, 1, sbs])

                # q content for both heads in one psum
                pc = psC.tile([128, SB], f32, tag="blk",
                              name=f"pc_{hp}_{sb}")
                for lc in range(NLC):
                    nc.tensor.matmul(
                        pc[:],
                        w_qc_t[:, lc, hp * 128:(hp + 1) * 128],
                        q_dT[:, lc, sbs],
                        start=(lc == 0), stop=(lc == NLC - 1))
                nc.vector.tensor_copy(qT[h0][0:64, sbs], pc[0:64, :])
                nc.vector.tensor_copy(qT[h1][0:32, sbs], pc[64:96, :])
                nc.vector.tensor_copy(qT[h1][32:64, sbs], pc[96:128, :])

                # B: v for this s-block (natural layout); pv feeds the
                # imminent AV matmuls, so keep its eviction fast on ACT
                for sc in range(4 * sb, 4 * sb + 4):
                    pv = psC.tile([128, SB], f32, tag="blk",
                                  name=f"pv_{hp}_{sc}")
                    for lc in range(NLC):
                        nc.tensor.matmul(
                            pv[:, 0:2 * HD],
                            kv_dT[:, lc, sc * 128:(sc + 1) * 128],
                            w_vu_t[:, lc, hp * 2 * HD:(hp + 1) * 2 * HD],
                            start=(lc == 0), stop=(lc == NLC - 1))
                    if sc % 2 == 0:
                        nc.scalar.copy(v_pair[hp][:, sc, :], pv[:, 0:2 * HD])
                    else:
                        nc.vector.tensor_copy(v_pair[hp][:, sc, :],
                                              pv[:, 0:2 * HD])

            def emit_c2(hp, qi):
                h0, h1 = 2 * hp, 2 * hp + 1
                nkj = (qi + 1) * dpq
                qs = slice(qi * SB, (qi + 1) * SB)
                py = {h: ps_y.tile([128, SB], f32, tag="py",
                                   name=f"py_{h}_{qi}")
                      for h in (h0, h1)}
                pden = {h: ps_b.tile([128, SB], f32, tag="pden",
                                     name=f"pden_{h}_{qi}")
                        for h in (h0, h1)}

                def score(h, kj, c0):
                    ps = psC.tile([128, SB], f32, tag="blk",
                                  name=f"ps_{h}_{qi}_{kj}")
                    nc.tensor.matmul(
                        ps[:, c0:SB], kT[h][:, kj * KB:(kj + 1) * KB],
                        qT[h][:, qi * SB + c0:(qi + 1) * SB],
                        start=True, stop=True)
                    return ps

                def finish(h, kj, c0, ps):
                    es = esp.tile([128, SB], mdt, tag="es",
                                  name=f"es_{h}_{qi}_{kj}")
                    nc.scalar.activation(es[:, c0:SB], ps[:, c0:SB],
                                         Exp, scale=SCALE)
                    if kj >= qi * dpq:          # diagonal block
                        nc.gpsimd.affine_select(
                            out=es[:, c0:c0 + KB], in_=es[:, c0:c0 + KB],
                            compare_op=is_ge, fill=0.0,
                            base=0, pattern=[[1, KB]],
                            channel_multiplier=-1)
                    nc.tensor.matmul(
                        py[h][:, c0:SB],
                        v_pair[hp][:, kj, (h - h0) * HD:(h - h0 + 1) * HD],
                        es[:, c0:SB], start=(kj == 0), stop=(kj == nkj - 1))
                    nc.tensor.matmul(
                        pden[h][:, c0:SB], ones128[:], es[:, c0:SB],
                        start=(kj == 0), stop=(kj == nkj - 1))

                pend = []
                for kj in range(nkj):
                    c0 = max(0, (kj - qi * dpq) * KB)
                    for h in (h0, h1):
                        pend.append((h, kj, c0, score(h, kj, c0)))
                        if len(pend) > 3:
                            finish(*pend.pop(0))
                for it in pend:
                    finish(*it)

                for h in (h0, h1):
                    # 1/den via the fast Newton-Raphson DVE op (~18
                    # correct bits; den is in [1, ~2e3] so no edge
                    # cases), then one DVE multiply (PSUM x SBUF)
                    rec = recp.tile([128, SB], f32, tag="rec",
                                    name=f"rec_{h}_{qi}")
                    nc.vector.reciprocal_approx_fast(
                        out=rec[:], in_=pden[h][:])
                    nc.vector.tensor_mul(yT_all[:, h, qs],
                                         py[h][:], rec[:])

            def emit_d(qi):
                # o-projection for q-block qi (all 4 heads' yT ready);
                # partials DMA straight from PSUM to DRAM - no compute
                # engine in the eviction path
                for sc in range(4 * qi, 4 * qi + 4):
                    for ncol in range(H // 512):
                        po = psC.tile([128, SB], f32, tag="blk",
                                      name=f"po_{sc}_{ncol}")
                        for hh in range(HPC):
                            nc.tensor.matmul(
                                po[:], yT_all[:, hh, sc * 128:(sc + 1) * 128],
                                wo_tiles[ncol][:, hh, :],
                                start=(hh == 0), stop=(hh == HPC - 1))
                        ot = outp.tile([128, 512], mdt, tag="ot")
                        nc.vector.tensor_copy(ot[:], po[:])
                        nc.sync.dma_start(
                            out[sc * 128:(sc + 1) * 128,
                                ncol * 512:(ncol + 1) * 512], ot[:])

            for sb in range(nsb):
                for hp in range(2):
                    emit_c1_b(hp, sb)
                if sb > 0:
                    emit_d(sb - 1)
                for hp in range(2):
                    emit_c2(hp, sb)
            emit_d(nsb - 1)

    nc.compile()
    return nc


# ======================= host-side preparation ==========================

def _np_dtype(mm_dtype):
    if mm_dtype == "bf16":
        import ml_dtypes

        return ml_dtypes.bfloat16
    return np.float32


def _rope_tables(s, ndt):
    inv_freq = 1.0 / (ROPE_BASE ** (np.arange(0, RD, 2, dtype=np.float64) / RD))
    t = np.arange(s, dtype=np.float64)
    freqs = np.outer(t, inv_freq)                    # [s, 32]
    cc = np.tile(np.cos(freqs).T, (4, 1)).astype(ndt)   # [128, s]
    ss = np.tile(np.sin(freqs).T, (4, 1)).astype(ndt)
    return np.ascontiguousarray(cc), np.ascontiguousarray(ss)


def make_in_maps(hidden_states, Wkv_d, Wq_d, Wk_u, Wq_u, Wv_u, Wrk, Wrq, Wo,
                 s=S, mm_dtype=MM_DTYPE):
    ndt = _np_dtype(mm_dtype)
    w_down = np.ascontiguousarray(
        np.concatenate([Wkv_d, Wq_d], axis=1), dtype=ndt)       # [H, 512]
    rope_cc, rope_ss = _rope_tables(s, ndt)
    ones = np.ones((128, 128), dtype=ndt)
    Wk_u4 = Wk_u.reshape(L, NH, RD)
    Wq_u4 = Wq_u.reshape(L, NH, RD)
    Wrq4 = Wrq.reshape(L, NH, RD)
    Wv_u4 = Wv_u.reshape(L, NH, HD)
    Wrk4 = Wrk.reshape(H, NH, RD)
    Wo4 = Wo.reshape(NH, HD, H)

    def pack_lo_hi(w4, hsel, dim0):
        # [dim0, 4 heads, 64] -> cols [h0_lo..h3_lo, h0_hi..h3_hi]
        wl = w4[:, hsel, 0:RD // 2]                  # [d, 4, 32]
        wh = w4[:, hsel, RD // 2:RD]
        return np.ascontiguousarray(np.concatenate(
            [wl.reshape(dim0, HPC * 32), wh.reshape(dim0, HPC * 32)],
            axis=1), dtype=ndt)                      # [d, 256]

    def pack_qr_pairs(w4, hsel):
        # per pair p: [h(2p)_lo, h(2p+1)_lo, h(2p)_hi, h(2p+1)_hi] (32 each)
        cols = []
        heads = list(range(hsel.start, hsel.stop))
        for p in range(2):
            ha, hb = heads[2 * p], heads[2 * p + 1]
            cols.extend([w4[:, ha, 0:32], w4[:, hb, 0:32],
                         w4[:, ha, 32:64], w4[:, hb, 32:64]])
        return np.ascontiguousarray(
            np.concatenate(cols, axis=1), dtype=ndt)  # [L, 256]

    hsT_b = [np.ascontiguousarray(hidden_states[b, :s].T, dtype=ndt)
             for b in range(B)]                       # [H, s] each

    in_maps = []
    for c in range(8):
        b, g = divmod(c, 4)
        hsel = slice(g * HPC, (g + 1) * HPC)
        in_maps.append({
            "hsT": hsT_b[b],
            "hsT_down": np.ascontiguousarray(hsT_b[b][:, g * SB:(g + 1) * SB]),
            "w_down": w_down,
            # k rope: all-lo then all-hi packing (matches k_rT chunks)
            "w_rk": pack_lo_hi(Wrk4, hsel, H),
            "w_qc": np.ascontiguousarray(
                Wq_u4[:, hsel, :].reshape(L, HPC * RD), dtype=ndt),
            "w_qr": pack_qr_pairs(Wrq4, hsel),
            "w_ku": np.ascontiguousarray(
                Wk_u4[:, hsel, :].reshape(L, HPC * RD), dtype=ndt),
            "w_vu": np.ascontiguousarray(
                Wv_u4[:, hsel, :].reshape(L, HPC * HD), dtype=ndt),
            "w_o": np.ascontiguousarray(
                Wo4[hsel].reshape(HPC * HD, H), dtype=ndt),
            "ones_in": ones,
            "rope_cc": rope_cc,
            "rope_ss": rope_ss,
        })
    return in_maps


_NC_CACHE = {}


def kernel(hidden_states, Wkv_d, Wq_d, Wk_u, Wq_u, Wv_u, Wrk, Wrq, Wo):
    from concourse.bass_utils import run_bass_kernel_spmd

    key = (S, MM_DTYPE)
    if key not in _NC_CACHE:
        _NC_CACHE[key] = build_nc(S, MM_DTYPE)
    nc = _NC_CACHE[key]

    in_maps = make_in_maps(
        np.asarray(hidden_states), np.asarray(Wkv_d), np.asarray(Wq_d),
        np.asarray(Wk_u), np.asarray(Wq_u), np.asarray(Wv_u),
        np.asarray(Wrk), np.asarray(Wrq), np.asarray(Wo))

    res = run_bass_kernel_spmd(nc, in_maps, core_ids=list(range(8)))
    parts = [r["out"].astype(np.float32) for r in res.results]
    out = np.empty((B, S, H), dtype=np.float32)
    for b in range(B):
        out[b] = parts[4 * b] + parts[4 * b + 1] + parts[4 * b + 2] + parts[4 * b + 3]
    return out


# revision 22
# speedup vs baseline: 1.2386x; 1.2386x over previous
"""Trainium2 Bass kernel for MultiHeadLatentAttention (MLA), 8-core SPMD.

Sharding: core c = (batch b=c//4, head-group g=c%4 of 4 heads).
Each core computes the full latent down-projections for its batch
(replicated across the 4 cores of that batch), head-sharded
up-projections + RoPE + causal attention, and a partial o-projection
(its heads' rows of Wo). The host sums the 4 partial outputs per batch.

Shapes (fixed): B=2, S=2048, H=2048, L=256, nh=16, hd=128, rd=64.

All matmul operands are bf16 (f32 PSUM accumulation); the host passes
hidden_states pre-transposed ([H, s], features on partitions) so the
device runs no transposes at all.

Device layouts (features on partitions):
  hsT         [128, NKC, s-block] streamed per s-block from DRAM
  kv_dT, q_dT [L=256 -> 2x128, s]
  k_rT        [2x128, s] chunk0 = all heads' rope-lo rows (4x32),
                         chunk1 = all heads' rope-hi rows; rotated in place
  qT_h, kT_h  [128 (64 content + 64 rope), s] per head
  v_all       [128 (s%128), s//128, 4 heads * 128]  (natural v)
  yT_all      [128 (hd), 4 heads, s]

Attention is computed in scores-transposed orientation S^T[k, q] so the
probabilities feed the AV matmul directly (lhsT = v block, rhs = expS).
Softmax skips the max-subtraction (scores are tiny here: |s| < ~2).
Causal structure: k-blocks strictly above the diagonal are skipped, and
diagonal k-blocks compute only the live column range, with a single
[128,128] affine_select for the triangular edge. The denominator is
accumulated per k-block by an all-ones matmul into PSUM (broadcasts it
across partitions); the reciprocal runs on the ACT engine straight from
PSUM and one vector multiply normalizes y.
"""

import os
import sys

sys.path.insert(0, "/opt/trn_rl_repo")

import numpy as np

B = 2
S = 2048
H = 2048
L = 256          # latent dim (2 chunks of 128)
NH = 16          # total heads
HPC = 4          # heads per core
HD = 128         # head dim
RD = 64          # rope / content half-width
ROPE_BASE = 10000.0
SCALE = float(HD) ** -0.5

SB = 512         # s-block for projections / q-blocks in attention
KB = 128         # k-block in attention
NKC = H // 128   # 16 contraction chunks over H
NLC = L // 128   # 2 contraction chunks over L

# Matmul input dtype: "bf16" (fast, ~2e-3 rel err) or "f32r" (~3e-4).
MM_DTYPE = os.environ.get("MLA_MM_DTYPE", "bf16")


def build_nc(s=S, mm_dtype=MM_DTYPE):
    """Build the Bass module for one core. `s` can be shrunk (multiple of 512)
    for simulator testing."""
    from concourse import bacc
    import concourse.bass as bass
    import concourse.mybir as mybir
    import concourse.tile as tile
    from contextlib import ExitStack

    f32 = mybir.dt.float32
    mdt = mybir.dt.bfloat16 if mm_dtype == "bf16" else mybir.dt.float32r

    nsb = s // SB        # s-blocks
    nsc = s // 128       # 128-row s-chunks

    nc = bacc.Bacc(None, target_bir_lowering=False)

    hsT = nc.dram_tensor("hsT", [H, s], mdt, kind="ExternalInput")
    w_down = nc.dram_tensor("w_down", [H, 512], mdt, kind="ExternalInput")
    w_rk = nc.dram_tensor("w_rk", [H, HPC * RD], mdt, kind="ExternalInput")
    w_qc = nc.dram_tensor("w_qc", [L, HPC * RD], mdt, kind="ExternalInput")
    w_qr = nc.dram_tensor("w_qr", [L, HPC * RD], mdt, kind="ExternalInput")
    w_ku = nc.dram_tensor("w_ku", [L, HPC * RD], mdt, kind="ExternalInput")
    w_vu = nc.dram_tensor("w_vu", [L, HPC * HD], mdt, kind="ExternalInput")
    w_o = nc.dram_tensor("w_o", [HPC * HD, H], mdt, kind="ExternalInput")
    ones_in = nc.dram_tensor("ones_in", [128, 128], mdt, kind="ExternalInput")
    # cos/sin halves replicated across all four 32-partition quadrants so any
    # 32-row operand can pair with a table slice at the SAME base partition
    # (walrus: both-SBUF tensor_tensor inputs must share base partition).
    rope_cc = nc.dram_tensor("rope_cc", [128, s], mdt, kind="ExternalInput")
    rope_ss = nc.dram_tensor("rope_ss", [128, s], mdt, kind="ExternalInput")
    out = nc.dram_tensor("out", [s, H], mdt, kind="ExternalOutput")

    Exp = mybir.ActivationFunctionType.Exp
    Ln = mybir.ActivationFunctionType.Ln
    is_ge = mybir.AluOpType.is_ge

    with ExitStack() as top:
        tc = top.enter_context(tile.TileContext(nc))

        # ---- persistent small pools -------------------------------------
        const_pool = top.enter_context(tc.tile_pool(name="const", bufs=1))
        ones128 = const_pool.tile([128, 128], mdt, tag="ones")
        cc_t = const_pool.tile([128, s], mdt, tag="ropec")
        ss_t = const_pool.tile([128, s], mdt, tag="ropes")

        wsmall = top.enter_context(tc.tile_pool(name="wsmall", bufs=1))
        w_qc_t = wsmall.tile([128, NLC, HPC * RD], mdt, tag="wqc")
        w_qr_t = wsmall.tile([128, NLC, HPC * RD], mdt, tag="wqr")
        w_ku_t = wsmall.tile([128, NLC, HPC * RD], mdt, tag="wku")
        w_vu_t = wsmall.tile([128, NLC, HPC * HD], mdt, tag="wvu")

        # ---- latent / rope-k tensors (live through all C1 phases) -------
        lat_pool = top.enter_context(tc.tile_pool(name="lat", bufs=1))
        kv_dT = lat_pool.tile([128, NLC, s], mdt, tag="kvd")   # [L, s]
        q_dT = lat_pool.tile([128, NLC, s], mdt, tag="qd")     # [L, s]
        k_rT = lat_pool.tile([128, NLC, s], mdt, tag="krt")    # rope k rows

        # ============ unified software pipeline over s-blocks ============
        # Per iteration sb: A(sb) latent/rope-k projections (two 3-stream
        # passes so A holds only 3 PSUM banks), k-rope rotation, C1(sb)
        # q/k/v head projections, D(sb-1) o-projection (independent PE
        # filler while C1's DVE chains drain), then C2(qi=sb) causal
        # attention.  PSUM: A 3 + shared C1/score/D ring 3 + py 2 = 8.
        # The softmax denominator is accumulated on DVE (es_acc) and
        # reduced by a single all-ones matmul per (head, q-block).
        hsT_r = hsT.rearrange("(ko p) m -> p ko m", p=128)

        vp = top.enter_context(tc.tile_pool(name="vp", bufs=1))
        yp = top.enter_context(tc.tile_pool(name="yp", bufs=1))
        yT_all = yp.tile([128, HPC, s], mdt, tag="yT")

        hstp = top.enter_context(tc.tile_pool(name="hst", bufs=2))
        wdp = top.enter_context(tc.tile_pool(name="wdown", bufs=1))
        wrkp = top.enter_context(tc.tile_pool(name="wrk", bufs=1))
        qkp = top.enter_context(tc.tile_pool(name="qkp", bufs=2))
        esp = top.enter_context(tc.tile_pool(name="esp", bufs=4))
        accp = top.enter_context(tc.tile_pool(name="accp", bufs=4))
        recp = top.enter_context(tc.tile_pool(name="recp", bufs=2))
        rtmp = top.enter_context(tc.tile_pool(name="rtmp", bufs=2))
        rkr = top.enter_context(tc.tile_pool(name="rkr", bufs=2))
        wop = top.enter_context(tc.tile_pool(name="wop", bufs=4))
        outp = top.enter_context(tc.tile_pool(name="outp", bufs=4))

        psA = top.enter_context(tc.tile_pool(name="psA", bufs=3, space="PSUM"))
        psC = top.enter_context(tc.tile_pool(name="psC", bufs=3, space="PSUM"))
        ps_y = top.enter_context(tc.tile_pool(name="ps_y", bufs=2, space="PSUM"))

        nqb = s // SB
        dpq = SB // KB                      # diagonal blocks per qi

        # weight / table DMAs, chunked so the first matmuls start early
        w_down_t = wdp.tile([128, NKC, 512], mdt, tag="wd")
        w_rk_t = wrkp.tile([128, NKC, HPC * RD], mdt, tag="wr")
        wd_r = w_down.rearrange("(ko p) m -> p ko m", p=128)
        wr_r = w_rk.rearrange("(ko p) m -> p ko m", p=128)
        for kc in range(NKC):
            nc.sync.dma_start(w_down_t[:, kc, :], wd_r[:, kc, :])
            nc.sync.dma_start(w_rk_t[:, kc, :], wr_r[:, kc, :])
        nc.sync.dma_start(ones128[:], ones_in[:])
        nc.sync.dma_start(cc_t[:], rope_cc[:])
        nc.sync.dma_start(ss_t[:], rope_ss[:])
        nc.sync.dma_start(
            w_qc_t[:], w_qc.rearrange("(ko p) m -> p ko m", p=128))
        nc.sync.dma_start(
            w_qr_t[:], w_qr.rearrange("(ko p) m -> p ko m", p=128))
        nc.sync.dma_start(
            w_ku_t[:], w_ku.rearrange("(ko p) m -> p ko m", p=128))
        nc.sync.dma_start(
            w_vu_t[:], w_vu.rearrange("(ko p) m -> p ko m", p=128))
        wo_tiles = []
        for ncol in range(H // 512):
            wo_t = wop.tile([128, HPC, 512], mdt, tag="wo",
                            name=f"wo_{ncol}")
            nc.sync.dma_start(
                wo_t[:],
                w_o[:, ncol * 512:(ncol + 1) * 512].rearrange(
                    "(ho p) m -> p ho m", p=128))
            wo_tiles.append(wo_t)

        hs_tiles = {}

        def load_hs(sb):
            if sb >= nsb:
                return
            t = hstp.tile([128, NKC, SB], mdt, tag="hsT", name=f"hsT_{sb}")
            for g in range(4):
                nc.gpsimd.dma_start(
                    t[:, 4 * g:4 * g + 4, :],
                    hsT_r[:, 4 * g:4 * g + 4, sb * SB:(sb + 1) * SB])
            hs_tiles[sb] = t

        qT = {h: qkp.tile([128, s], mdt, tag="qT", bufs=4,
                          name=f"qT_{h}") for h in range(2 * HPC // 2)}
        kT = {h: qkp.tile([128, s], mdt, tag="kT", bufs=4,
                          name=f"kT_{h}") for h in range(2 * HPC // 2)}
        v_pair = {hp: vp.tile([128, nsc, 2 * HD], mdt, tag="vpair",
                              bufs=2, name=f"v_pair_{hp}")
                  for hp in range(2)}

        def emit_a(sb):
            # latent + rope-k projections for s-block sb, two 3-stream
            # passes over the same hsT tile (3 PSUM banks)
            sbs = slice(sb * SB, (sb + 1) * SB)
            hsTt = hs_tiles.pop(sb)
            passes = (
                (("kv0", 0, kv_dT, 0), ("kv1", 128, kv_dT, 1),
                 ("q0", 256, q_dT, 0)),
                (("q1", 384, q_dT, 1), ("kr0", 512, k_rT, 0),
                 ("kr1", 640, k_rT, 1)),
            )
            for pi, streams in enumerate(passes):
                pb = {}
                for n, off, _, _ in streams:
                    pb[n] = psA.tile([128, SB], f32, tag="pa",
                                     name=f"pa_{n}_{sb}")
                for kc in range(NKC):
                    rhs = hsTt[:, kc, :]
                    st = kc == 0
                    sp = kc == NKC - 1
                    for n, off, _, _ in streams:
                        if off < 512:
                            w = w_down_t[:, kc, off:off + 128]
                        else:
                            w = w_rk_t[:, kc, off - 512:off - 384]
                        nc.tensor.matmul(pb[n][:], w, rhs, start=st, stop=sp)
                for i, (n, off, dst, lc) in enumerate(streams):
                    if i % 2 == 0:
                        nc.scalar.copy(dst[:, lc, sbs], pb[n][:])
                    else:
                        nc.vector.tensor_copy(dst[:, lc, sbs], pb[n][:])

        def emit_rot(sb):
            # rotate k_rT in place (RoPE on all 4 heads at once):
            # chunk0 rows = all heads' lo dims, chunk1 = hi dims:
            #   lo' = lo*cos - hi*sin ;  hi' = hi*cos + lo*sin
            sbs = slice(sb * SB, (sb + 1) * SB)
            r1 = rkr.tile([128, SB], mdt, tag="r1")
            r2 = rkr.tile([128, SB], mdt, tag="r2")
            r3 = rkr.tile([128, SB], mdt, tag="r3")
            r4 = rkr.tile([128, SB], mdt, tag="r4")
            lo = k_rT[:, 0, sbs]
            hi = k_rT[:, 1, sbs]
            nc.vector.tensor_mul(r1[:], lo, cc_t[:, sbs])
            nc.vector.tensor_mul(r2[:], hi, ss_t[:, sbs])
            nc.vector.tensor_mul(r3[:], hi, cc_t[:, sbs])
            nc.vector.tensor_mul(r4[:], lo, ss_t[:, sbs])
            nc.vector.tensor_sub(k_rT[:, 0, sbs], r1[:], r2[:])
            nc.vector.tensor_add(k_rT[:, 1, sbs], r3[:], r4[:])

        def emit_c1_b(hp, sb):
            h0, h1 = 2 * hp, 2 * hp + 1
            sbs = slice(sb * SB, (sb + 1) * SB)
            # q rope FIRST: its DVE chain is the longest pole to the
            # first score matmul of this q-block.
            # psum rows = [h0_lo, h1_lo, h0_hi, h1_hi] (32 each)
            pr = psC.tile([128, SB], f32, tag="blk",
                          name=f"pr_{hp}_{sb}")
            for lc in range(NLC):
                nc.tensor.matmul(
                    pr[:],
                    w_qr_t[:, lc, hp * 128:(hp + 1) * 128],
                    q_dT[:, lc, sbs],
                    start=(lc == 0), stop=(lc == NLC - 1))
            t1 = rtmp.tile([64, SB], mdt, tag="t1")
            t2 = rtmp.tile([64, SB], mdt, tag="t2")
            t3 = rtmp.tile([64, SB], mdt, tag="t3")
            t4 = rtmp.tile([64, SB], mdt, tag="t4")
            nc.vector.tensor_mul(t1[:], pr[0:64, :], cc_t[0:64, sbs])
            nc.vector.tensor_mul(t2[:], pr[64:128, :], ss_t[64:128, sbs])
            nc.vector.tensor_mul(t3[:], pr[64:128, :], cc_t[64:128, sbs])
            nc.vector.tensor_mul(t4[:], pr[0:64, :], ss_t[0:64, sbs])
            # write rotated rows straight into each head's qT
            nc.vector.tensor_sub(qT[h0][64:96, sbs],
                                 t1[0:32, :], t2[0:32, :])
            nc.vector.tensor_sub(qT[h1][64:96, sbs],
                                 t1[32:64, :], t2[32:64, :])
            nc.vector.tensor_add(qT[h0][96:128, sbs],
                                 t3[0:32, :], t4[0:32, :])
            nc.vector.tensor_add(qT[h1][96:128, sbs],
                                 t3[32:64, :], t4[32:64, :])

            # k content for both heads in one [128, SB] psum.  All
            # evictions on DVE so ACT stays nearly exp-only (the exp
            # feeds the AV matmul critical path).
            pk = psC.tile([128, SB], f32, tag="blk",
                          name=f"pk_{hp}_{sb}")
            for lc in range(NLC):
                nc.tensor.matmul(
                    pk[:],
                    w_ku_t[:, lc, hp * 128:(hp + 1) * 128],
                    kv_dT[:, lc, sbs],
                    start=(lc == 0), stop=(lc == NLC - 1))
            nc.vector.tensor_copy(kT[h0][0:64, sbs], pk[0:64, :])
            nc.vector.tensor_copy(kT[h1][0:32, sbs], pk[64:96, :])
            nc.vector.tensor_copy(kT[h1][32:64, sbs], pk[96:128, :])
            # k rope: copy pre-rotated k_rT rows on DVE
            for h in (h0, h1):
                rb = slice(32 * h, 32 * h + 32)
                nc.vector.tensor_copy(kT[h][64:96, sbs],
                                      k_rT[rb, 0, sbs])
                nc.vector.tensor_copy(kT[h][96:128, sbs],
                                      k_rT[rb, 1, sbs])

            # q content for both heads in one psum
            pc = psC.tile([128, SB], f32, tag="blk",
                          name=f"pc_{hp}_{sb}")
            for lc in range(NLC):
                nc.tensor.matmul(
                    pc[:],
                    w_qc_t[:, lc, hp * 128:(hp + 1) * 128],
                    q_dT[:, lc, sbs],
                    start=(lc == 0), stop=(lc == NLC - 1))
            nc.vector.tensor_copy(qT[h0][0:64, sbs], pc[0:64, :])
            nc.vector.tensor_copy(qT[h1][0:32, sbs], pc[64:96, :])
            nc.vector.tensor_copy(qT[h1][32:64, sbs], pc[96:128, :])

            # B: v for this s-block (natural layout); pv feeds the
            # imminent AV matmuls, so keep its eviction fast on ACT
            for sc in range(4 * sb, 4 * sb + 4):
                pv = psC.tile([128, SB], f32, tag="blk",
                              name=f"pv_{hp}_{sc}")
                for lc in range(NLC):
                    nc.tensor.matmul(
                        pv[:, 0:2 * HD],
                        kv_dT[:, lc, sc * 128:(sc + 1) * 128],
                        w_vu_t[:, lc, hp * 2 * HD:(hp + 1) * 2 * HD],
                        start=(lc == 0), stop=(lc == NLC - 1))
                if sc % 2 == 0:
                    nc.scalar.copy(v_pair[hp][:, sc, :], pv[:, 0:2 * HD])
                else:
                    nc.vector.tensor_copy(v_pair[hp][:, sc, :],
                                          pv[:, 0:2 * HD])

        def emit_c2(hp, qi):
            h0, h1 = 2 * hp, 2 * hp + 1
            nkj = (qi + 1) * dpq
            qs = slice(qi * SB, (qi + 1) * SB)
            py = {h: ps_y.tile([128, SB], f32, tag="py",
                               name=f"py_{h}_{qi}")
                  for h in (h0, h1)}
            acc = {h: accp.tile([128, SB], mdt, tag="acc",
                                name=f"acc_{h}_{qi}")
                   for h in (h0, h1)}

            def score(h, kj, c0):
                ps = psC.tile([128, SB], f32, tag="blk",
                              name=f"ps_{h}_{qi}_{kj}")
                nc.tensor.matmul(
                    ps[:, c0:SB], kT[h][:, kj * KB:(kj + 1) * KB],
                    qT[h][:, qi * SB + c0:(qi + 1) * SB],
                    start=True, stop=True)
                return ps

            def finish(h, kj, c0, ps):
                es = esp.tile([128, SB], mdt, tag="es",
                              name=f"es_{h}_{qi}_{kj}")
                nc.scalar.activation(es[:, c0:SB], ps[:, c0:SB],
                                     Exp, scale=SCALE)
                if kj >= qi * dpq:          # diagonal block
                    nc.gpsimd.affine_select(
                        out=es[:, c0:c0 + KB], in_=es[:, c0:c0 + KB],
                        compare_op=is_ge, fill=0.0,
                        base=0, pattern=[[1, KB]],
                        channel_multiplier=-1)
                nc.tensor.matmul(
                    py[h][:, c0:SB],
                    v_pair[hp][:, kj, (h - h0) * HD:(h - h0 + 1) * HD],
                    es[:, c0:SB], start=(kj == 0), stop=(kj == nkj - 1))
                # denominator accumulation on DVE (kj=0 is always full
                # width, so the copy initializes the whole tile)
                if kj == 0:
                    nc.vector.tensor_copy(acc[h][:], es[:])
                else:
                    nc.vector.tensor_add(acc[h][:, c0:SB],
                                         acc[h][:, c0:SB], es[:, c0:SB])

            pend = []
            for kj in range(nkj):
                c0 = max(0, (kj - qi * dpq) * KB)
                for h in (h0, h1):
                    pend.append((h, kj, c0, score(h, kj, c0)))
                    if len(pend) > 3:
                        finish(*pend.pop(0))
            for it in pend:
                finish(*it)

            for h in (h0, h1):
                # partition-reduce the denominator with one all-ones
                # matmul (broadcasts den across partitions), then 1/den
                # via the fast Newton-Raphson DVE op (~18 correct bits;
                # den is in [1, ~2e3] so no edge cases) and one DVE
                # multiply (PSUM x SBUF)
                pden = psC.tile([128, SB], f32, tag="blk",
                                name=f"pden_{h}_{qi}")
                nc.tensor.matmul(pden[:], ones128[:], acc[h][:],
                                 start=True, stop=True)
                rec = recp.tile([128, SB], f32, tag="rec",
                                name=f"rec_{h}_{qi}")
                nc.vector.reciprocal_approx_fast(
                    out=rec[:], in_=pden[:])
                nc.vector.tensor_mul(yT_all[:, h, qs],
                                     py[h][:], rec[:])

        def emit_d(qi):
            # o-projection for q-block qi (all 4 heads' yT ready)
            for sc in range(4 * qi, 4 * qi + 4):
                for ncol in range(H // 512):
                    po = psC.tile([128, SB], f32, tag="blk",
                                  name=f"po_{sc}_{ncol}")
                    for hh in range(HPC):
                        nc.tensor.matmul(
                            po[:], yT_all[:, hh, sc * 128:(sc + 1) * 128],
                            wo_tiles[ncol][:, hh, :],
                            start=(hh == 0), stop=(hh == HPC - 1))
                    ot = outp.tile([128, 512], mdt, tag="ot")
                    nc.vector.tensor_copy(ot[:], po[:])
                    nc.sync.dma_start(
                        out[sc * 128:(sc + 1) * 128,
                            ncol * 512:(ncol + 1) * 512], ot[:])

        load_hs(0)
        load_hs(1)
        for sb in range(nsb):
            emit_a(sb)
            load_hs(sb + 2)
            emit_rot(sb)
            for hp in range(2):
                emit_c1_b(hp, sb)
            if sb > 0:
                emit_d(sb - 1)
            for hp in range(2):
                emit_c2(hp, sb)
        emit_d(nsb - 1)

    nc.compile()
    return nc


# ======================= host-side preparation ==========================

def _np_dtype(mm_dtype):
    if mm_dtype == "bf16":
        import ml_dtypes

        return ml_dtypes.bfloat16
    return np.float32


def _rope_tables(s, ndt):
    inv_freq = 1.0 / (ROPE_BASE ** (np.arange(0, RD, 2, dtype=np.float64) / RD))
    t = np.arange(s, dtype=np.float64)
    freqs = np.outer(t, inv_freq)                    # [s, 32]
    cc = np.tile(np.cos(freqs).T, (4, 1)).astype(ndt)   # [128, s]
    ss = np.tile(np.sin(freqs).T, (4, 1)).astype(ndt)
    return np.ascontiguousarray(cc), np.ascontiguousarray(ss)


def make_in_maps(hidden_states, Wkv_d, Wq_d, Wk_u, Wq_u, Wv_u, Wrk, Wrq, Wo,
                 s=S, mm_dtype=MM_DTYPE):
    ndt = _np_dtype(mm_dtype)
    w_down = np.ascontiguousarray(
        np.concatenate([Wkv_d, Wq_d], axis=1), dtype=ndt)       # [H, 512]
    rope_cc, rope_ss = _rope_tables(s, ndt)
    ones = np.ones((128, 128), dtype=ndt)
    Wk_u4 = Wk_u.reshape(L, NH, RD)
    Wq_u4 = Wq_u.reshape(L, NH, RD)
    Wrq4 = Wrq.reshape(L, NH, RD)
    Wv_u4 = Wv_u.reshape(L, NH, HD)
    Wrk4 = Wrk.reshape(H, NH, RD)
    Wo4 = Wo.reshape(NH, HD, H)

    def pack_lo_hi(w4, hsel, dim0):
        # [dim0, 4 heads, 64] -> cols [h0_lo..h3_lo, h0_hi..h3_hi]
        wl = w4[:, hsel, 0:RD // 2]                  # [d, 4, 32]
        wh = w4[:, hsel, RD // 2:RD]
        return np.ascontiguousarray(np.concatenate(
            [wl.reshape(dim0, HPC * 32), wh.reshape(dim0, HPC * 32)],
            axis=1), dtype=ndt)                      # [d, 256]

    def pack_qr_pairs(w4, hsel):
        # per pair p: [h(2p)_lo, h(2p+1)_lo, h(2p)_hi, h(2p+1)_hi] (32 each)
        cols = []
        heads = list(range(hsel.start, hsel.stop))
        for p in range(2):
            ha, hb = heads[2 * p], heads[2 * p + 1]
            cols.extend([w4[:, ha, 0:32], w4[:, hb, 0:32],
                         w4[:, ha, 32:64], w4[:, hb, 32:64]])
        return np.ascontiguousarray(
            np.concatenate(cols, axis=1), dtype=ndt)  # [L, 256]

    hsT_b = [np.ascontiguousarray(hidden_states[b, :s].T, dtype=ndt)
             for b in range(B)]                       # [H, s] each

    in_maps = []
    for c in range(8):
        b, g = divmod(c, 4)
        hsel = slice(g * HPC, (g + 1) * HPC)
        in_maps.append({
            "hsT": hsT_b[b],
            "w_down": w_down,
            # k rope: all-lo then all-hi packing (matches k_rT chunks)
            "w_rk": pack_lo_hi(Wrk4, hsel, H),
            "w_qc": np.ascontiguousarray(
                Wq_u4[:, hsel, :].reshape(L, HPC * RD), dtype=ndt),
            "w_qr": pack_qr_pairs(Wrq4, hsel),
            "w_ku": np.ascontiguousarray(
                Wk_u4[:, hsel, :].reshape(L, HPC * RD), dtype=ndt),
            "w_vu": np.ascontiguousarray(
                Wv_u4[:, hsel, :].reshape(L, HPC * HD), dtype=ndt),
            "w_o": np.ascontiguousarray(
                Wo4[hsel].reshape(HPC * HD, H), dtype=ndt),
            "ones_in": ones,
            "rope_cc": rope_cc,
            "rope_ss": rope_ss,
        })
    return in_maps


_NC_CACHE = {}


def kernel(hidden_states, Wkv_d, Wq_d, Wk_u, Wq_u, Wv_u, Wrk, Wrq, Wo):
    from concourse.bass_utils import run_bass_kernel_spmd

    key = (S, MM_DTYPE)
    if key not in _NC_CACHE:
        _NC_CACHE[key] = build_nc(S, MM_DTYPE)
    nc = _NC_CACHE[key]

    in_maps = make_in_maps(
        np.asarray(hidden_states), np.asarray(Wkv_d), np.asarray(Wq_d),
        np.asarray(Wk_u), np.asarray(Wq_u), np.asarray(Wv_u),
        np.asarray(Wrk), np.asarray(Wrq), np.asarray(Wo))

    res = run_bass_kernel_spmd(nc, in_maps, core_ids=list(range(8)))
    parts = [r["out"].astype(np.float32) for r in res.results]
    out = np.empty((B, S, H), dtype=np.float32)
    for b in range(B):
        out[b] = parts[4 * b] + parts[4 * b + 1] + parts[4 * b + 2] + parts[4 * b + 3]
    return out


# revision 25
# speedup vs baseline: 1.2436x; 1.0040x over previous
"""Trainium2 Bass kernel for MultiHeadLatentAttention (MLA), 8-core SPMD.

Sharding: core c = (batch b=c//4, head-group g=c%4 of 4 heads).
Each core computes the full latent down-projections for its batch
(replicated across the 4 cores of that batch), head-sharded
up-projections + RoPE + causal attention, and a partial o-projection
(its heads' rows of Wo). The host sums the 4 partial outputs per batch.

Shapes (fixed): B=2, S=2048, H=2048, L=256, nh=16, hd=128, rd=64.

All matmul operands are bf16 (f32 PSUM accumulation); the host passes
hidden_states pre-transposed ([H, s], features on partitions) so the
device runs no transposes at all.

Device layouts (features on partitions):
  hsT         [128, NKC, s-block] streamed per s-block from DRAM
  kv_dT, q_dT [L=256 -> 2x128, s]
  k_rT        [2x128, s] chunk0 = all heads' rope-lo rows (4x32),
                         chunk1 = all heads' rope-hi rows; rotated in place
  qT_h, kT_h  [128 (64 content + 64 rope), s] per head
  v_all       [128 (s%128), s//128, 4 heads * 128]  (natural v)
  yT_all      [128 (hd), 4 heads, s]

Attention is computed in scores-transposed orientation S^T[k, q] so the
probabilities feed the AV matmul directly (lhsT = v block, rhs = expS).
Softmax skips the max-subtraction (scores are tiny here: |s| < ~2).
Causal structure: k-blocks strictly above the diagonal are skipped, and
diagonal k-blocks compute only the live column range, with a single
[128,128] affine_select for the triangular edge. The denominator is
accumulated per k-block by an all-ones matmul into PSUM (broadcasts it
across partitions); the reciprocal runs on the ACT engine straight from
PSUM and one vector multiply normalizes y.
"""

import os
import sys

sys.path.insert(0, "/opt/trn_rl_repo")

import numpy as np

B = 2
S = 2048
H = 2048
L = 256          # latent dim (2 chunks of 128)
NH = 16          # total heads
HPC = 4          # heads per core
HD = 128         # head dim
RD = 64          # rope / content half-width
ROPE_BASE = 10000.0
SCALE = float(HD) ** -0.5

SB = 512         # s-block for projections / q-blocks in attention
KB = 128         # k-block in attention
NKC = H // 128   # 16 contraction chunks over H
NLC = L // 128   # 2 contraction chunks over L

# Matmul input dtype: "bf16" (fast, ~2e-3 rel err) or "f32r" (~3e-4).
MM_DTYPE = os.environ.get("MLA_MM_DTYPE", "bf16")


def build_nc(s=S, mm_dtype=MM_DTYPE):
    """Build the Bass module for one core. `s` can be shrunk (multiple of 512)
    for simulator testing."""
    from concourse import bacc
    import concourse.bass as bass
    import concourse.mybir as mybir
    import concourse.tile as tile
    from contextlib import ExitStack

    f32 = mybir.dt.float32
    mdt = mybir.dt.bfloat16 if mm_dtype == "bf16" else mybir.dt.float32r

    nsb = s // SB        # s-blocks
    nsc = s // 128       # 128-row s-chunks

    nc = bacc.Bacc(None, target_bir_lowering=False)

    hsT = nc.dram_tensor("hsT", [H, s], mdt, kind="ExternalInput")
    w_down = nc.dram_tensor("w_down", [H, 512], mdt, kind="ExternalInput")
    w_rk = nc.dram_tensor("w_rk", [H, HPC * RD], mdt, kind="ExternalInput")
    w_qc = nc.dram_tensor("w_qc", [L, HPC * RD], mdt, kind="ExternalInput")
    w_qr = nc.dram_tensor("w_qr", [L, HPC * RD], mdt, kind="ExternalInput")
    w_ku = nc.dram_tensor("w_ku", [L, HPC * RD], mdt, kind="ExternalInput")
    w_vu = nc.dram_tensor("w_vu", [L, HPC * HD], mdt, kind="ExternalInput")
    w_o = nc.dram_tensor("w_o", [HPC * HD, H], mdt, kind="ExternalInput")
    ones_in = nc.dram_tensor("ones_in", [128, 128], mdt, kind="ExternalInput")
    # cos/sin halves replicated across all four 32-partition quadrants so any
    # 32-row operand can pair with a table slice at the SAME base partition
    # (walrus: both-SBUF tensor_tensor inputs must share base partition).
    rope_cc = nc.dram_tensor("rope_cc", [128, s], mdt, kind="ExternalInput")
    rope_ss = nc.dram_tensor("rope_ss", [128, s], mdt, kind="ExternalInput")
    out = nc.dram_tensor("out", [s, H], mdt, kind="ExternalOutput")

    Exp = mybir.ActivationFunctionType.Exp
    Ln = mybir.ActivationFunctionType.Ln
    is_ge = mybir.AluOpType.is_ge

    with ExitStack() as top:
        tc = top.enter_context(tile.TileContext(nc))

        # ---- persistent small pools -------------------------------------
        const_pool = top.enter_context(tc.tile_pool(name="const", bufs=1))
        ones128 = const_pool.tile([128, 128], mdt, tag="ones")
        cc_t = const_pool.tile([128, s], mdt, tag="ropec")
        ss_t = const_pool.tile([128, s], mdt, tag="ropes")

        wsmall = top.enter_context(tc.tile_pool(name="wsmall", bufs=1))
        w_qc_t = wsmall.tile([128, NLC, HPC * RD], mdt, tag="wqc")
        w_qr_t = wsmall.tile([128, NLC, HPC * RD], mdt, tag="wqr")
        w_ku_t = wsmall.tile([128, NLC, HPC * RD], mdt, tag="wku")
        w_vu_t = wsmall.tile([128, NLC, HPC * HD], mdt, tag="wvu")

        # ---- latent / rope-k tensors (live through all C1 phases) -------
        lat_pool = top.enter_context(tc.tile_pool(name="lat", bufs=1))
        kv_dT = lat_pool.tile([128, NLC, s], mdt, tag="kvd")   # [L, s]
        q_dT = lat_pool.tile([128, NLC, s], mdt, tag="qd")     # [L, s]
        k_rT = lat_pool.tile([128, NLC, s], mdt, tag="krt")    # rope k rows

        # ============ unified software pipeline over s-blocks ============
        # Per iteration sb: A(sb) latent/rope-k projections (two 3-stream
        # passes so A holds only 3 PSUM banks), k-rope rotation, C1(sb)
        # q/k/v head projections, D(sb-1) o-projection (independent PE
        # filler while C1's DVE chains drain), then C2(qi=sb) causal
        # attention.  PSUM: A 3 + shared C1/score/D ring 3 + py 2 = 8.
        # The softmax denominator is accumulated on DVE (es_acc) and
        # reduced by a single all-ones matmul per (head, q-block).
        hsT_r = hsT.rearrange("(ko p) m -> p ko m", p=128)

        vp = top.enter_context(tc.tile_pool(name="vp", bufs=1))
        yp = top.enter_context(tc.tile_pool(name="yp", bufs=1))
        yT_all = yp.tile([128, HPC, s], mdt, tag="yT")

        hstp = top.enter_context(tc.tile_pool(name="hst", bufs=2))
        wdp = top.enter_context(tc.tile_pool(name="wdown", bufs=1))
        wrkp = top.enter_context(tc.tile_pool(name="wrk", bufs=1))
        qkp = top.enter_context(tc.tile_pool(name="qkp", bufs=2))
        esp = top.enter_context(tc.tile_pool(name="esp", bufs=4))
        accp = top.enter_context(tc.tile_pool(name="accp", bufs=4))
        recp = top.enter_context(tc.tile_pool(name="recp", bufs=2))
        rtmp = top.enter_context(tc.tile_pool(name="rtmp", bufs=2))
        rkr = top.enter_context(tc.tile_pool(name="rkr", bufs=2))
        wop = top.enter_context(tc.tile_pool(name="wop", bufs=4))
        outp = top.enter_context(tc.tile_pool(name="outp", bufs=4))

        psA = top.enter_context(tc.tile_pool(name="psA", bufs=3, space="PSUM"))
        psC = top.enter_context(tc.tile_pool(name="psC", bufs=3, space="PSUM"))
        ps_y = top.enter_context(tc.tile_pool(name="ps_y", bufs=2, space="PSUM"))

        nqb = s // SB
        dpq = SB // KB                      # diagonal blocks per qi

        # weight / table DMAs, chunked so the first matmuls start early
        w_down_t = wdp.tile([128, NKC, 512], mdt, tag="wd")
        w_rk_t = wrkp.tile([128, NKC, HPC * RD], mdt, tag="wr")
        wd_r = w_down.rearrange("(ko p) m -> p ko m", p=128)
        wr_r = w_rk.rearrange("(ko p) m -> p ko m", p=128)
        for kc in range(NKC):
            nc.sync.dma_start(w_down_t[:, kc, :], wd_r[:, kc, :])
            nc.sync.dma_start(w_rk_t[:, kc, :], wr_r[:, kc, :])
        nc.sync.dma_start(ones128[:], ones_in[:])
        nc.sync.dma_start(cc_t[:], rope_cc[:])
        nc.sync.dma_start(ss_t[:], rope_ss[:])
        nc.sync.dma_start(
            w_qc_t[:], w_qc.rearrange("(ko p) m -> p ko m", p=128))
        nc.sync.dma_start(
            w_qr_t[:], w_qr.rearrange("(ko p) m -> p ko m", p=128))
        nc.sync.dma_start(
            w_ku_t[:], w_ku.rearrange("(ko p) m -> p ko m", p=128))
        nc.sync.dma_start(
            w_vu_t[:], w_vu.rearrange("(ko p) m -> p ko m", p=128))
        hs_tiles = {}

        def load_hs(sb):
            if sb >= nsb:
                return
            t = hstp.tile([128, NKC, SB], mdt, tag="hsT", name=f"hsT_{sb}")
            for g in range(4):
                nc.gpsimd.dma_start(
                    t[:, 4 * g:4 * g + 4, :],
                    hsT_r[:, 4 * g:4 * g + 4, sb * SB:(sb + 1) * SB])
            hs_tiles[sb] = t

        qT = {h: qkp.tile([128, s], mdt, tag="qT", bufs=4,
                          name=f"qT_{h}") for h in range(2 * HPC // 2)}
        kT = {h: qkp.tile([128, s], mdt, tag="kT", bufs=4,
                          name=f"kT_{h}") for h in range(2 * HPC // 2)}
        v_pair = {hp: vp.tile([128, nsc, 2 * HD], mdt, tag="vpair",
                              bufs=2, name=f"v_pair_{hp}")
                  for hp in range(2)}

        def emit_a(sb):
            # latent + rope-k projections for s-block sb, two 3-stream
            # passes over the same hsT tile (3 PSUM banks)
            sbs = slice(sb * SB, (sb + 1) * SB)
            hsTt = hs_tiles.pop(sb)
            passes = (
                (("kv0", 0, kv_dT, 0), ("kv1", 128, kv_dT, 1),
                 ("q0", 256, q_dT, 0)),
                (("q1", 384, q_dT, 1), ("kr0", 512, k_rT, 0),
                 ("kr1", 640, k_rT, 1)),
            )
            for pi, streams in enumerate(passes):
                pb = {}
                for n, off, _, _ in streams:
                    pb[n] = psA.tile([128, SB], f32, tag="pa",
                                     name=f"pa_{n}_{sb}")
                for kc in range(NKC):
                    rhs = hsTt[:, kc, :]
                    st = kc == 0
                    sp = kc == NKC - 1
                    for n, off, _, _ in streams:
                        if off < 512:
                            w = w_down_t[:, kc, off:off + 128]
                        else:
                            w = w_rk_t[:, kc, off - 512:off - 384]
                        nc.tensor.matmul(pb[n][:], w, rhs, start=st, stop=sp)
                for i, (n, off, dst, lc) in enumerate(streams):
                    if i % 2 == 0:
                        nc.scalar.copy(dst[:, lc, sbs], pb[n][:])
                    else:
                        nc.vector.tensor_copy(dst[:, lc, sbs], pb[n][:])

        def emit_rot(sb):
            # rotate k_rT in place (RoPE on all 4 heads at once):
            # chunk0 rows = all heads' lo dims, chunk1 = hi dims:
            #   lo' = lo*cos - hi*sin ;  hi' = hi*cos + lo*sin
            sbs = slice(sb * SB, (sb + 1) * SB)
            r1 = rkr.tile([128, SB], mdt, tag="r1")
            r2 = rkr.tile([128, SB], mdt, tag="r2")
            r3 = rkr.tile([128, SB], mdt, tag="r3")
            r4 = rkr.tile([128, SB], mdt, tag="r4")
            lo = k_rT[:, 0, sbs]
            hi = k_rT[:, 1, sbs]
            nc.vector.tensor_mul(r1[:], lo, cc_t[:, sbs])
            nc.vector.tensor_mul(r2[:], hi, ss_t[:, sbs])
            nc.vector.tensor_mul(r3[:], hi, cc_t[:, sbs])
            nc.vector.tensor_mul(r4[:], lo, ss_t[:, sbs])
            nc.vector.tensor_sub(k_rT[:, 0, sbs], r1[:], r2[:])
            nc.vector.tensor_add(k_rT[:, 1, sbs], r3[:], r4[:])

        def emit_c1_b(hp, sb):
            h0, h1 = 2 * hp, 2 * hp + 1
            sbs = slice(sb * SB, (sb + 1) * SB)
            # q rope FIRST: its DVE chain is the longest pole to the
            # first score matmul of this q-block.
            # psum rows = [h0_lo, h1_lo, h0_hi, h1_hi] (32 each)
            pr = psC.tile([128, SB], f32, tag="blk",
                          name=f"pr_{hp}_{sb}")
            for lc in range(NLC):
                nc.tensor.matmul(
                    pr[:],
                    w_qr_t[:, lc, hp * 128:(hp + 1) * 128],
                    q_dT[:, lc, sbs],
                    start=(lc == 0), stop=(lc == NLC - 1))
            t1 = rtmp.tile([64, SB], mdt, tag="t1")
            t2 = rtmp.tile([64, SB], mdt, tag="t2")
            t3 = rtmp.tile([64, SB], mdt, tag="t3")
            t4 = rtmp.tile([64, SB], mdt, tag="t4")
            nc.vector.tensor_mul(t1[:], pr[0:64, :], cc_t[0:64, sbs])
            nc.vector.tensor_mul(t2[:], pr[64:128, :], ss_t[64:128, sbs])
            nc.vector.tensor_mul(t3[:], pr[64:128, :], cc_t[64:128, sbs])
            nc.vector.tensor_mul(t4[:], pr[0:64, :], ss_t[0:64, sbs])
            # write rotated rows straight into each head's qT
            nc.vector.tensor_sub(qT[h0][64:96, sbs],
                                 t1[0:32, :], t2[0:32, :])
            nc.vector.tensor_sub(qT[h1][64:96, sbs],
                                 t1[32:64, :], t2[32:64, :])
            nc.vector.tensor_add(qT[h0][96:128, sbs],
                                 t3[0:32, :], t4[0:32, :])
            nc.vector.tensor_add(qT[h1][96:128, sbs],
                                 t3[32:64, :], t4[32:64, :])

            # k content for both heads in one [128, SB] psum.  All
            # evictions on DVE so ACT stays nearly exp-only (the exp
            # feeds the AV matmul critical path).
            pk = psC.tile([128, SB], f32, tag="blk",
                          name=f"pk_{hp}_{sb}")
            for lc in range(NLC):
                nc.tensor.matmul(
                    pk[:],
                    w_ku_t[:, lc, hp * 128:(hp + 1) * 128],
                    kv_dT[:, lc, sbs],
                    start=(lc == 0), stop=(lc == NLC - 1))
            nc.vector.tensor_copy(kT[h0][0:64, sbs], pk[0:64, :])
            nc.vector.tensor_copy(kT[h1][0:32, sbs], pk[64:96, :])
            nc.vector.tensor_copy(kT[h1][32:64, sbs], pk[96:128, :])
            # k rope: copy pre-rotated k_rT rows on DVE
            for h in (h0, h1):
                rb = slice(32 * h, 32 * h + 32)
                nc.vector.tensor_copy(kT[h][64:96, sbs],
                                      k_rT[rb, 0, sbs])
                nc.vector.tensor_copy(kT[h][96:128, sbs],
                                      k_rT[rb, 1, sbs])

            # q content for both heads in one psum
            pc = psC.tile([128, SB], f32, tag="blk",
                          name=f"pc_{hp}_{sb}")
            for lc in range(NLC):
                nc.tensor.matmul(
                    pc[:],
                    w_qc_t[:, lc, hp * 128:(hp + 1) * 128],
                    q_dT[:, lc, sbs],
                    start=(lc == 0), stop=(lc == NLC - 1))
            nc.vector.tensor_copy(qT[h0][0:64, sbs], pc[0:64, :])
            nc.vector.tensor_copy(qT[h1][0:32, sbs], pc[64:96, :])
            nc.vector.tensor_copy(qT[h1][32:64, sbs], pc[96:128, :])

            # B: v for this s-block (natural layout); pv feeds the
            # imminent AV matmuls, so keep its eviction fast on ACT
            for sc in range(4 * sb, 4 * sb + 4):
                pv = psC.tile([128, SB], f32, tag="blk",
                              name=f"pv_{hp}_{sc}")
                for lc in range(NLC):
                    nc.tensor.matmul(
                        pv[:, 0:2 * HD],
                        kv_dT[:, lc, sc * 128:(sc + 1) * 128],
                        w_vu_t[:, lc, hp * 2 * HD:(hp + 1) * 2 * HD],
                        start=(lc == 0), stop=(lc == NLC - 1))
                nc.scalar.copy(v_pair[hp][:, sc, :], pv[:, 0:2 * HD])

        def emit_c2(hp, qi):
            h0, h1 = 2 * hp, 2 * hp + 1
            nkj = (qi + 1) * dpq
            qs = slice(qi * SB, (qi + 1) * SB)
            py = {h: ps_y.tile([128, SB], f32, tag="py",
                               name=f"py_{h}_{qi}")
                  for h in (h0, h1)}
            acc = {h: accp.tile([128, SB], mdt, tag="acc",
                                name=f"acc_{h}_{qi}")
                   for h in (h0, h1)}

            def score(h, kj, c0):
                ps = psC.tile([128, SB], f32, tag="blk",
                              name=f"ps_{h}_{qi}_{kj}")
                nc.tensor.matmul(
                    ps[:, c0:SB], kT[h][:, kj * KB:(kj + 1) * KB],
                    qT[h][:, qi * SB + c0:(qi + 1) * SB],
                    start=True, stop=True)
                return ps

            def finish(h, kj, c0, ps):
                es = esp.tile([128, SB], mdt, tag="es",
                              name=f"es_{h}_{qi}_{kj}")
                nc.scalar.activation(es[:, c0:SB], ps[:, c0:SB],
                                     Exp, scale=SCALE)
                if kj >= qi * dpq:          # diagonal block
                    nc.gpsimd.affine_select(
                        out=es[:, c0:c0 + KB], in_=es[:, c0:c0 + KB],
                        compare_op=is_ge, fill=0.0,
                        base=0, pattern=[[1, KB]],
                        channel_multiplier=-1)
                nc.tensor.matmul(
                    py[h][:, c0:SB],
                    v_pair[hp][:, kj, (h - h0) * HD:(h - h0 + 1) * HD],
                    es[:, c0:SB], start=(kj == 0), stop=(kj == nkj - 1))
                # denominator accumulation on DVE (kj=0 is always full
                # width, so the copy initializes the whole tile)
                if kj == 0:
                    nc.vector.tensor_copy(acc[h][:], es[:])
                else:
                    nc.vector.tensor_add(acc[h][:, c0:SB],
                                         acc[h][:, c0:SB], es[:, c0:SB])

            pend = []
            for kj in range(nkj):
                c0 = max(0, (kj - qi * dpq) * KB)
                for h in (h0, h1):
                    pend.append((h, kj, c0, score(h, kj, c0)))
                    if len(pend) > 3:
                        finish(*pend.pop(0))
            for it in pend:
                finish(*it)

            for h in (h0, h1):
                # partition-reduce the denominator with one all-ones
                # matmul (broadcasts den across partitions), then 1/den
                # via the fast Newton-Raphson DVE op (~18 correct bits;
                # den is in [1, ~2e3] so no edge cases) and one DVE
                # multiply (PSUM x SBUF)
                pden = psC.tile([128, SB], f32, tag="blk",
                                name=f"pden_{h}_{qi}")
                nc.tensor.matmul(pden[:], ones128[:], acc[h][:],
                                 start=True, stop=True)
                rec = recp.tile([128, SB], f32, tag="rec",
                                name=f"rec_{h}_{qi}")
                nc.vector.reciprocal_approx_fast(
                    out=rec[:], in_=pden[:])
                nc.vector.tensor_mul(yT_all[:, h, qs],
                                     py[h][:], rec[:])

        def emit_d(qi):
            # o-projection for q-block qi (all 4 heads' yT ready)
            for sc in range(4 * qi, 4 * qi + 4):
                for ncol in range(H // 512):
                    po = psC.tile([128, SB], f32, tag="blk",
                                  name=f"po_{sc}_{ncol}")
                    for hh in range(HPC):
                        nc.tensor.matmul(
                            po[:], yT_all[:, hh, sc * 128:(sc + 1) * 128],
                            wo_tiles[ncol][:, hh, :],
                            start=(hh == 0), stop=(hh == HPC - 1))
                    ot = outp.tile([128, 512], mdt, tag="ot")
                    if ncol % 2 == 0:
                        nc.scalar.copy(ot[:], po[:])
                    else:
                        nc.vector.tensor_copy(ot[:], po[:])
                    nc.sync.dma_start(
                        out[sc * 128:(sc + 1) * 128,
                            ncol * 512:(ncol + 1) * 512], ot[:])

        load_hs(0)
        load_hs(1)
        # o-proj weights after the first two hsT blocks on the same queue:
        # off the critical startup window, well before first use (~110us)
        wo_tiles = []
        for ncol in range(H // 512):
            wo_t = wop.tile([128, HPC, 512], mdt, tag="wo",
                            name=f"wo_{ncol}")
            nc.gpsimd.dma_start(
                wo_t[:],
                w_o[:, ncol * 512:(ncol + 1) * 512].rearrange(
                    "(ho p) m -> p ho m", p=128))
            wo_tiles.append(wo_t)
        for sb in range(nsb):
            emit_a(sb)
            load_hs(sb + 2)
            emit_rot(sb)
            for hp in range(2):
                emit_c1_b(hp, sb)
            if sb > 0:
                emit_d(sb - 1)
            for hp in range(2):
                emit_c2(hp, sb)
        emit_d(nsb - 1)

    nc.compile()
    return nc


# ======================= host-side preparation ==========================

def _np_dtype(mm_dtype):
    if mm_dtype == "bf16":
        import ml_dtypes

        return ml_dtypes.bfloat16
    return np.float32


def _rope_tables(s, ndt):
    inv_freq = 1.0 / (ROPE_BASE ** (np.arange(0, RD, 2, dtype=np.float64) / RD))
    t = np.arange(s, dtype=np.float64)
    freqs = np.outer(t, inv_freq)                    # [s, 32]
    cc = np.tile(np.cos(freqs).T, (4, 1)).astype(ndt)   # [128, s]
    ss = np.tile(np.sin(freqs).T, (4, 1)).astype(ndt)
    return np.ascontiguousarray(cc), np.ascontiguousarray(ss)


def make_in_maps(hidden_states, Wkv_d, Wq_d, Wk_u, Wq_u, Wv_u, Wrk, Wrq, Wo,
                 s=S, mm_dtype=MM_DTYPE):
    ndt = _np_dtype(mm_dtype)
    w_down = np.ascontiguousarray(
        np.concatenate([Wkv_d, Wq_d], axis=1), dtype=ndt)       # [H, 512]
    rope_cc, rope_ss = _rope_tables(s, ndt)
    ones = np.ones((128, 128), dtype=ndt)
    Wk_u4 = Wk_u.reshape(L, NH, RD)
    Wq_u4 = Wq_u.reshape(L, NH, RD)
    Wrq4 = Wrq.reshape(L, NH, RD)
    Wv_u4 = Wv_u.reshape(L, NH, HD)
    Wrk4 = Wrk.reshape(H, NH, RD)
    Wo4 = Wo.reshape(NH, HD, H)

    def pack_lo_hi(w4, hsel, dim0):
        # [dim0, 4 heads, 64] -> cols [h0_lo..h3_lo, h0_hi..h3_hi]
        wl = w4[:, hsel, 0:RD // 2]                  # [d, 4, 32]
        wh = w4[:, hsel, RD // 2:RD]
        return np.ascontiguousarray(np.concatenate(
            [wl.reshape(dim0, HPC * 32), wh.reshape(dim0, HPC * 32)],
            axis=1), dtype=ndt)                      # [d, 256]

    def pack_qr_pairs(w4, hsel):
        # per pair p: [h(2p)_lo, h(2p+1)_lo, h(2p)_hi, h(2p+1)_hi] (32 each)
        cols = []
        heads = list(range(hsel.start, hsel.stop))
        for p in range(2):
            ha, hb = heads[2 * p], heads[2 * p + 1]
            cols.extend([w4[:, ha, 0:32], w4[:, hb, 0:32],
                         w4[:, ha, 32:64], w4[:, hb, 32:64]])
        return np.ascontiguousarray(
            np.concatenate(cols, axis=1), dtype=ndt)  # [L, 256]

    hsT_b = [np.ascontiguousarray(hidden_states[b, :s].T, dtype=ndt)
             for b in range(B)]                       # [H, s] each

    in_maps = []
    for c in range(8):
        b, g = divmod(c, 4)
        hsel = slice(g * HPC, (g + 1) * HPC)
        in_maps.append({
            "hsT": hsT_b[b],
            "w_down": w_down,
            # k rope: all-lo then all-hi packing (matches k_rT chunks)
            "w_rk": pack_lo_hi(Wrk4, hsel, H),
            "w_qc": np.ascontiguousarray(
                Wq_u4[:, hsel, :].reshape(L, HPC * RD), dtype=ndt),
            "w_qr": pack_qr_pairs(Wrq4, hsel),
            "w_ku": np.ascontiguousarray(
                Wk_u4[:, hsel, :].reshape(L, HPC * RD), dtype=ndt),
            "w_vu": np.ascontiguousarray(
                Wv_u4[:, hsel, :].reshape(L, HPC * HD), dtype=ndt),
            "w_o": np.ascontiguousarray(
                Wo4[hsel].reshape(HPC * HD, H), dtype=ndt),
            "ones_in": ones,
            "rope_cc": rope_cc,
            "rope_ss": rope_ss,
        })
    return in_maps


_NC_CACHE = {}


def kernel(hidden_states, Wkv_d, Wq_d, Wk_u, Wq_u, Wv_u, Wrk, Wrq, Wo):
    from concourse.bass_utils import run_bass_kernel_spmd

    key = (S, MM_DTYPE)
    if key not in _NC_CACHE:
        _NC_CACHE[key] = build_nc(S, MM_DTYPE)
    nc = _NC_CACHE[key]

    in_maps = make_in_maps(
        np.asarray(hidden_states), np.asarray(Wkv_d), np.asarray(Wq_d),
        np.asarray(Wk_u), np.asarray(Wq_u), np.asarray(Wv_u),
        np.asarray(Wrk), np.asarray(Wrq), np.asarray(Wo))

    res = run_bass_kernel_spmd(nc, in_maps, core_ids=list(range(8)))
    parts = [r["out"].astype(np.float32) for r in res.results]
    out = np.empty((B, S, H), dtype=np.float32)
    for b in range(B):
        out[b] = parts[4 * b] + parts[4 * b + 1] + parts[4 * b + 2] + parts[4 * b + 3]
    return out


# revision 26
# speedup vs baseline: 1.3128x; 1.0556x over previous
"""Trainium2 Bass kernel for MultiHeadLatentAttention (MLA), 8-core SPMD.

Sharding: core c = (batch b=c//4, head-group g=c%4 of 4 heads).
Each core computes the full latent down-projections for its batch
(replicated across the 4 cores of that batch), head-sharded
up-projections + RoPE + causal attention, and a partial o-projection
(its heads' rows of Wo). The host sums the 4 partial outputs per batch.

Shapes (fixed): B=2, S=2048, H=2048, L=256, nh=16, hd=128, rd=64.

All matmul operands are bf16 (f32 PSUM accumulation); the host passes
hidden_states pre-transposed ([H, s], features on partitions) so the
device runs no transposes at all.

Device layouts (features on partitions):
  hsT         [128, NKC, s-block] streamed per s-block from DRAM
  kv_dT, q_dT [L=256 -> 2x128, s]
  k_rT        [2x128, s] chunk0 = all heads' rope-lo rows (4x32),
                         chunk1 = all heads' rope-hi rows; rotated in place
  qT_h, kT_h  [128 (64 content + 64 rope), s] per head
  v_all       [128 (s%128), s//128, 4 heads * 128]  (natural v)
  yT_all      [128 (hd), 4 heads, s]

Attention is computed in scores-transposed orientation S^T[k, q] so the
probabilities feed the AV matmul directly (lhsT = v block, rhs = expS).
Softmax skips the max-subtraction (scores are tiny here: |s| < ~2).
Causal structure: k-blocks strictly above the diagonal are skipped, and
diagonal k-blocks compute only the live column range, with a single
[128,128] affine_select for the triangular edge. The denominator is
accumulated per k-block by an all-ones matmul into PSUM (broadcasts it
across partitions); the reciprocal runs on the ACT engine straight from
PSUM and one vector multiply normalizes y.
"""

import os
import sys

sys.path.insert(0, "/opt/trn_rl_repo")

import numpy as np

B = 2
S = 2048
H = 2048
L = 256          # latent dim (2 chunks of 128)
NH = 16          # total heads
HPC = 4          # heads per core
HD = 128         # head dim
RD = 64          # rope / content half-width
ROPE_BASE = 10000.0
SCALE = float(HD) ** -0.5

SB = 512         # s-block for projections / q-blocks in attention
KB = 128         # k-block in attention
NKC = H // 128   # 16 contraction chunks over H
NLC = L // 128   # 2 contraction chunks over L

# Matmul input dtype: "bf16" (fast, ~2e-3 rel err) or "f32r" (~3e-4).
MM_DTYPE = os.environ.get("MLA_MM_DTYPE", "bf16")


def build_nc(s=S, mm_dtype=MM_DTYPE):
    """Build the Bass module for one core. `s` can be shrunk (multiple of 512)
    for simulator testing."""
    from concourse import bacc
    import concourse.bass as bass
    import concourse.mybir as mybir
    import concourse.tile as tile
    from contextlib import ExitStack

    f32 = mybir.dt.float32
    mdt = mybir.dt.bfloat16 if mm_dtype == "bf16" else mybir.dt.float32r

    nsb = s // SB        # s-blocks
    nsc = s // 128       # 128-row s-chunks

    nc = bacc.Bacc(None, target_bir_lowering=False)

    hsT = nc.dram_tensor("hsT", [H, s], mdt, kind="ExternalInput")
    w_down = nc.dram_tensor("w_down", [H, 512], mdt, kind="ExternalInput")
    w_rk = nc.dram_tensor("w_rk", [H, HPC * RD], mdt, kind="ExternalInput")
    w_qc = nc.dram_tensor("w_qc", [L, HPC * RD], mdt, kind="ExternalInput")
    w_qr = nc.dram_tensor("w_qr", [L, HPC * RD], mdt, kind="ExternalInput")
    w_ku = nc.dram_tensor("w_ku", [L, HPC * RD], mdt, kind="ExternalInput")
    w_vu = nc.dram_tensor("w_vu", [L, HPC * HD], mdt, kind="ExternalInput")
    w_o = nc.dram_tensor("w_o", [HPC * HD, H], mdt, kind="ExternalInput")
    ones_in = nc.dram_tensor("ones_in", [128, 128], mdt, kind="ExternalInput")
    # cos/sin halves replicated across all four 32-partition quadrants so any
    # 32-row operand can pair with a table slice at the SAME base partition
    # (walrus: both-SBUF tensor_tensor inputs must share base partition).
    rope_cc = nc.dram_tensor("rope_cc", [128, s], mdt, kind="ExternalInput")
    rope_ss = nc.dram_tensor("rope_ss", [128, s], mdt, kind="ExternalInput")
    out = nc.dram_tensor("out", [s, H], mdt, kind="ExternalOutput")

    Exp = mybir.ActivationFunctionType.Exp
    Ln = mybir.ActivationFunctionType.Ln
    is_ge = mybir.AluOpType.is_ge

    with ExitStack() as top:
        tc = top.enter_context(tile.TileContext(nc))

        # ---- persistent small pools -------------------------------------
        const_pool = top.enter_context(tc.tile_pool(name="const", bufs=1))
        ones128 = const_pool.tile([128, 128], mdt, tag="ones")
        cc_t = const_pool.tile([128, s], mdt, tag="ropec")
        ss_t = const_pool.tile([128, s], mdt, tag="ropes")

        wsmall = top.enter_context(tc.tile_pool(name="wsmall", bufs=1))
        w_qc_t = wsmall.tile([128, NLC, HPC * RD], mdt, tag="wqc")
        w_qr_t = wsmall.tile([128, NLC, HPC * RD], mdt, tag="wqr")
        w_ku_t = wsmall.tile([128, NLC, HPC * RD], mdt, tag="wku")
        w_vu_t = wsmall.tile([128, NLC, HPC * HD], mdt, tag="wvu")

        # ---- latent / rope-k tensors (live through all C1 phases) -------
        lat_pool = top.enter_context(tc.tile_pool(name="lat", bufs=1))
        kv_dT = lat_pool.tile([128, NLC, s], mdt, tag="kvd")   # [L, s]
        q_dT = lat_pool.tile([128, NLC, s], mdt, tag="qd")     # [L, s]
        k_rT = lat_pool.tile([128, NLC, s], mdt, tag="krt")    # rope k rows

        # ============ unified software pipeline over s-blocks ============
        # Per iteration sb: A(sb) latent/rope-k projections (two 3-stream
        # passes so A holds only 3 PSUM banks), k-rope rotation, C1(sb)
        # q/k/v head projections, D(sb-1) o-projection (independent PE
        # filler while C1's DVE chains drain), then C2(qi=sb) causal
        # attention.  PSUM: A 3 + shared C1/score/D ring 3 + py 2 = 8.
        # The softmax denominator is accumulated on DVE (es_acc) and
        # reduced by a single all-ones matmul per (head, q-block).
        hsT_r = hsT.rearrange("(ko p) m -> p ko m", p=128)

        vp = top.enter_context(tc.tile_pool(name="vp", bufs=1))
        yp = top.enter_context(tc.tile_pool(name="yp", bufs=1))
        yT_all = yp.tile([128, HPC, s], mdt, tag="yT")

        hstp = top.enter_context(tc.tile_pool(name="hst", bufs=2))
        wdp = top.enter_context(tc.tile_pool(name="wdown", bufs=1))
        wrkp = top.enter_context(tc.tile_pool(name="wrk", bufs=1))
        qkp = top.enter_context(tc.tile_pool(name="qkp", bufs=2))
        esp = top.enter_context(tc.tile_pool(name="esp", bufs=4))
        accp = top.enter_context(tc.tile_pool(name="accp", bufs=4))
        recp = top.enter_context(tc.tile_pool(name="recp", bufs=2))
        rtmp = top.enter_context(tc.tile_pool(name="rtmp", bufs=2))
        rkr = top.enter_context(tc.tile_pool(name="rkr", bufs=2))
        wop = top.enter_context(tc.tile_pool(name="wop", bufs=4))
        outp = top.enter_context(tc.tile_pool(name="outp", bufs=4))

        psA = top.enter_context(tc.tile_pool(name="psA", bufs=3, space="PSUM"))
        psC = top.enter_context(tc.tile_pool(name="psC", bufs=3, space="PSUM"))
        ps_y = top.enter_context(tc.tile_pool(name="ps_y", bufs=2, space="PSUM"))

        nqb = s // SB
        dpq = SB // KB                      # diagonal blocks per qi

        # weight / table DMAs, chunked so the first matmuls start early
        w_down_t = wdp.tile([128, NKC, 512], mdt, tag="wd")
        w_rk_t = wrkp.tile([128, NKC, HPC * RD], mdt, tag="wr")
        wd_r = w_down.rearrange("(ko p) m -> p ko m", p=128)
        wr_r = w_rk.rearrange("(ko p) m -> p ko m", p=128)
        for kc in range(NKC):
            nc.sync.dma_start(w_down_t[:, kc, :], wd_r[:, kc, :])
            nc.sync.dma_start(w_rk_t[:, kc, :], wr_r[:, kc, :])
        nc.sync.dma_start(ones128[:], ones_in[:])
        nc.sync.dma_start(cc_t[:], rope_cc[:])
        nc.sync.dma_start(ss_t[:], rope_ss[:])
        nc.sync.dma_start(
            w_qc_t[:], w_qc.rearrange("(ko p) m -> p ko m", p=128))
        nc.sync.dma_start(
            w_qr_t[:], w_qr.rearrange("(ko p) m -> p ko m", p=128))
        nc.sync.dma_start(
            w_ku_t[:], w_ku.rearrange("(ko p) m -> p ko m", p=128))
        nc.sync.dma_start(
            w_vu_t[:], w_vu.rearrange("(ko p) m -> p ko m", p=128))
        hs_tiles = {}

        def load_hs(sb):
            if sb >= nsb:
                return
            t = hstp.tile([128, NKC, SB], mdt, tag="hsT", name=f"hsT_{sb}")
            for g in range(4):
                nc.gpsimd.dma_start(
                    t[:, 4 * g:4 * g + 4, :],
                    hsT_r[:, 4 * g:4 * g + 4, sb * SB:(sb + 1) * SB])
            hs_tiles[sb] = t

        qT = {h: qkp.tile([128, s], mdt, tag="qT", bufs=4,
                          name=f"qT_{h}") for h in range(2 * HPC // 2)}
        kT = {h: qkp.tile([128, s], mdt, tag="kT", bufs=4,
                          name=f"kT_{h}") for h in range(2 * HPC // 2)}
        v_pair = {hp: vp.tile([128, nsc, 2 * HD], mdt, tag="vpair",
                              bufs=2, name=f"v_pair_{hp}")
                  for hp in range(2)}

        def emit_a(sb):
            # latent + rope-k projections for s-block sb, two 3-stream
            # passes over the same hsT tile (3 PSUM banks)
            sbs = slice(sb * SB, (sb + 1) * SB)
            hsTt = hs_tiles.pop(sb)
            passes = (
                (("kv0", 0, kv_dT, 0), ("kv1", 128, kv_dT, 1),
                 ("q0", 256, q_dT, 0)),
                (("q1", 384, q_dT, 1), ("kr0", 512, k_rT, 0),
                 ("kr1", 640, k_rT, 1)),
            )
            for pi, streams in enumerate(passes):
                pb = {}
                for n, off, _, _ in streams:
                    pb[n] = psA.tile([128, SB], f32, tag="pa",
                                     name=f"pa_{n}_{sb}")
                for kc in range(NKC):
                    rhs = hsTt[:, kc, :]
                    st = kc == 0
                    sp = kc == NKC - 1
                    for n, off, _, _ in streams:
                        if off < 512:
                            w = w_down_t[:, kc, off:off + 128]
                        else:
                            w = w_rk_t[:, kc, off - 512:off - 384]
                        nc.tensor.matmul(pb[n][:], w, rhs, start=st, stop=sp)
                for i, (n, off, dst, lc) in enumerate(streams):
                    if i % 2 == 0:
                        nc.scalar.copy(dst[:, lc, sbs], pb[n][:])
                    else:
                        nc.vector.tensor_copy(dst[:, lc, sbs], pb[n][:])

        def emit_rot(sb):
            # rotate k_rT in place (RoPE on all 4 heads at once):
            # chunk0 rows = all heads' lo dims, chunk1 = hi dims:
            #   lo' = lo*cos - hi*sin ;  hi' = hi*cos + lo*sin
            sbs = slice(sb * SB, (sb + 1) * SB)
            r1 = rkr.tile([128, SB], mdt, tag="r1")
            r2 = rkr.tile([128, SB], mdt, tag="r2")
            r3 = rkr.tile([128, SB], mdt, tag="r3")
            r4 = rkr.tile([128, SB], mdt, tag="r4")
            lo = k_rT[:, 0, sbs]
            hi = k_rT[:, 1, sbs]
            nc.vector.tensor_mul(r1[:], lo, cc_t[:, sbs])
            nc.vector.tensor_mul(r2[:], hi, ss_t[:, sbs])
            nc.vector.tensor_mul(r3[:], hi, cc_t[:, sbs])
            nc.vector.tensor_mul(r4[:], lo, ss_t[:, sbs])
            nc.vector.tensor_sub(k_rT[:, 0, sbs], r1[:], r2[:])
            nc.vector.tensor_add(k_rT[:, 1, sbs], r3[:], r4[:])

        def emit_c1_b(hp, sb):
            h0, h1 = 2 * hp, 2 * hp + 1
            sbs = slice(sb * SB, (sb + 1) * SB)
            # q rope FIRST: its DVE chain is the longest pole to the
            # first score matmul of this q-block.
            # psum rows = [h0_lo, h1_lo, h0_hi, h1_hi] (32 each)
            pr = psC.tile([128, SB], f32, tag="blk",
                          name=f"pr_{hp}_{sb}")
            for lc in range(NLC):
                nc.tensor.matmul(
                    pr[:],
                    w_qr_t[:, lc, hp * 128:(hp + 1) * 128],
                    q_dT[:, lc, sbs],
                    start=(lc == 0), stop=(lc == NLC - 1))
            t1 = rtmp.tile([64, SB], mdt, tag="t1")
            t2 = rtmp.tile([64, SB], mdt, tag="t2")
            t3 = rtmp.tile([64, SB], mdt, tag="t3")
            t4 = rtmp.tile([64, SB], mdt, tag="t4")
            nc.vector.tensor_mul(t1[:], pr[0:64, :], cc_t[0:64, sbs])
            nc.vector.tensor_mul(t2[:], pr[64:128, :], ss_t[64:128, sbs])
            nc.vector.tensor_mul(t3[:], pr[64:128, :], cc_t[64:128, sbs])
            nc.vector.tensor_mul(t4[:], pr[0:64, :], ss_t[0:64, sbs])
            # write rotated rows straight into each head's qT
            nc.vector.tensor_sub(qT[h0][64:96, sbs],
                                 t1[0:32, :], t2[0:32, :])
            nc.vector.tensor_sub(qT[h1][64:96, sbs],
                                 t1[32:64, :], t2[32:64, :])
            nc.vector.tensor_add(qT[h0][96:128, sbs],
                                 t3[0:32, :], t4[0:32, :])
            nc.vector.tensor_add(qT[h1][96:128, sbs],
                                 t3[32:64, :], t4[32:64, :])

            # k content for both heads in one [128, SB] psum.  All
            # evictions on DVE so ACT stays nearly exp-only (the exp
            # feeds the AV matmul critical path).
            pk = psC.tile([128, SB], f32, tag="blk",
                          name=f"pk_{hp}_{sb}")
            for lc in range(NLC):
                nc.tensor.matmul(
                    pk[:],
                    w_ku_t[:, lc, hp * 128:(hp + 1) * 128],
                    kv_dT[:, lc, sbs],
                    start=(lc == 0), stop=(lc == NLC - 1))
            nc.scalar.copy(kT[h0][0:64, sbs], pk[0:64, :])
            nc.scalar.copy(kT[h1][0:32, sbs], pk[64:96, :])
            nc.scalar.copy(kT[h1][32:64, sbs], pk[96:128, :])
            # k rope: copy pre-rotated k_rT rows on DVE
            for h in (h0, h1):
                rb = slice(32 * h, 32 * h + 32)
                nc.vector.tensor_copy(kT[h][64:96, sbs],
                                      k_rT[rb, 0, sbs])
                nc.vector.tensor_copy(kT[h][96:128, sbs],
                                      k_rT[rb, 1, sbs])

            # q content for both heads in one psum
            pc = psC.tile([128, SB], f32, tag="blk",
                          name=f"pc_{hp}_{sb}")
            for lc in range(NLC):
                nc.tensor.matmul(
                    pc[:],
                    w_qc_t[:, lc, hp * 128:(hp + 1) * 128],
                    q_dT[:, lc, sbs],
                    start=(lc == 0), stop=(lc == NLC - 1))
            nc.scalar.copy(qT[h0][0:64, sbs], pc[0:64, :])
            nc.scalar.copy(qT[h1][0:32, sbs], pc[64:96, :])
            nc.scalar.copy(qT[h1][32:64, sbs], pc[96:128, :])

            # B: v for this s-block (natural layout); pv feeds the
            # imminent AV matmuls, so keep its eviction fast on ACT
            for sc in range(4 * sb, 4 * sb + 4):
                pv = psC.tile([128, SB], f32, tag="blk",
                              name=f"pv_{hp}_{sc}")
                for lc in range(NLC):
                    nc.tensor.matmul(
                        pv[:, 0:2 * HD],
                        kv_dT[:, lc, sc * 128:(sc + 1) * 128],
                        w_vu_t[:, lc, hp * 2 * HD:(hp + 1) * 2 * HD],
                        start=(lc == 0), stop=(lc == NLC - 1))
                nc.scalar.copy(v_pair[hp][:, sc, :], pv[:, 0:2 * HD])

        def emit_c2(hp, qi):
            h0, h1 = 2 * hp, 2 * hp + 1
            nkj = (qi + 1) * dpq
            qs = slice(qi * SB, (qi + 1) * SB)
            py = {h: ps_y.tile([128, SB], f32, tag="py",
                               name=f"py_{h}_{qi}")
                  for h in (h0, h1)}
            acc = {h: accp.tile([128, SB], mdt, tag="acc",
                                name=f"acc_{h}_{qi}")
                   for h in (h0, h1)}

            def score(h, kj, c0):
                ps = psC.tile([128, SB], f32, tag="blk",
                              name=f"ps_{h}_{qi}_{kj}")
                nc.tensor.matmul(
                    ps[:, c0:SB], kT[h][:, kj * KB:(kj + 1) * KB],
                    qT[h][:, qi * SB + c0:(qi + 1) * SB],
                    start=True, stop=True)
                return ps

            def finish(h, kj, c0, ps):
                es = esp.tile([128, SB], mdt, tag="es",
                              name=f"es_{h}_{qi}_{kj}")
                nc.scalar.activation(es[:, c0:SB], ps[:, c0:SB],
                                     Exp, scale=SCALE)
                if kj >= qi * dpq:          # diagonal block
                    nc.gpsimd.affine_select(
                        out=es[:, c0:c0 + KB], in_=es[:, c0:c0 + KB],
                        compare_op=is_ge, fill=0.0,
                        base=0, pattern=[[1, KB]],
                        channel_multiplier=-1)
                nc.tensor.matmul(
                    py[h][:, c0:SB],
                    v_pair[hp][:, kj, (h - h0) * HD:(h - h0 + 1) * HD],
                    es[:, c0:SB], start=(kj == 0), stop=(kj == nkj - 1))
                # denominator accumulation on DVE (kj=0 is always full
                # width, so the copy initializes the whole tile)
                if kj == 0:
                    nc.vector.tensor_copy(acc[h][:], es[:])
                else:
                    nc.vector.tensor_add(acc[h][:, c0:SB],
                                         acc[h][:, c0:SB], es[:, c0:SB])

            pend = []
            for kj in range(nkj):
                c0 = max(0, (kj - qi * dpq) * KB)
                for h in (h0, h1):
                    pend.append((h, kj, c0, score(h, kj, c0)))
                    if len(pend) > 3:
                        finish(*pend.pop(0))
            for it in pend:
                finish(*it)

            for h in (h0, h1):
                # partition-reduce the denominator with one all-ones
                # matmul (broadcasts den across partitions), then 1/den
                # via the fast Newton-Raphson DVE op (~18 correct bits;
                # den is in [1, ~2e3] so no edge cases) and one DVE
                # multiply (PSUM x SBUF)
                pden = psC.tile([128, SB], f32, tag="blk",
                                name=f"pden_{h}_{qi}")
                nc.tensor.matmul(pden[:], ones128[:], acc[h][:],
                                 start=True, stop=True)
                rec = recp.tile([128, SB], f32, tag="rec",
                                name=f"rec_{h}_{qi}")
                nc.vector.reciprocal_approx_fast(
                    out=rec[:], in_=pden[:])
                nc.vector.tensor_mul(yT_all[:, h, qs],
                                     py[h][:], rec[:])

        def emit_d(qi):
            # o-projection for q-block qi (all 4 heads' yT ready)
            for sc in range(4 * qi, 4 * qi + 4):
                for ncol in range(H // 512):
                    po = psC.tile([128, SB], f32, tag="blk",
                                  name=f"po_{sc}_{ncol}")
                    for hh in range(HPC):
                        nc.tensor.matmul(
                            po[:], yT_all[:, hh, sc * 128:(sc + 1) * 128],
                            wo_tiles[ncol][:, hh, :],
                            start=(hh == 0), stop=(hh == HPC - 1))
                    ot = outp.tile([128, 512], mdt, tag="ot")
                    if ncol % 2 == 0:
                        nc.scalar.copy(ot[:], po[:])
                    else:
                        nc.vector.tensor_copy(ot[:], po[:])
                    nc.sync.dma_start(
                        out[sc * 128:(sc + 1) * 128,
                            ncol * 512:(ncol + 1) * 512], ot[:])

        load_hs(0)
        load_hs(1)
        # o-proj weights after the first two hsT blocks on the same queue:
        # off the critical startup window, well before first use (~110us)
        wo_tiles = []
        for ncol in range(H // 512):
            wo_t = wop.tile([128, HPC, 512], mdt, tag="wo",
                            name=f"wo_{ncol}")
            nc.gpsimd.dma_start(
                wo_t[:],
                w_o[:, ncol * 512:(ncol + 1) * 512].rearrange(
                    "(ho p) m -> p ho m", p=128))
            wo_tiles.append(wo_t)
        for sb in range(nsb):
            emit_a(sb)
            load_hs(sb + 2)
            emit_rot(sb)
            for hp in range(2):
                emit_c1_b(hp, sb)
            if sb > 0:
                emit_d(sb - 1)
            for hp in range(2):
                emit_c2(hp, sb)
        emit_d(nsb - 1)

    nc.compile()
    return nc


# ======================= host-side preparation ==========================

def _np_dtype(mm_dtype):
    if mm_dtype == "bf16":
        import ml_dtypes

        return ml_dtypes.bfloat16
    return np.float32


def _rope_tables(s, ndt):
    inv_freq = 1.0 / (ROPE_BASE ** (np.arange(0, RD, 2, dtype=np.float64) / RD))
    t = np.arange(s, dtype=np.float64)
    freqs = np.outer(t, inv_freq)                    # [s, 32]
    cc = np.tile(np.cos(freqs).T, (4, 1)).astype(ndt)   # [128, s]
    ss = np.tile(np.sin(freqs).T, (4, 1)).astype(ndt)
    return np.ascontiguousarray(cc), np.ascontiguousarray(ss)


def make_in_maps(hidden_states, Wkv_d, Wq_d, Wk_u, Wq_u, Wv_u, Wrk, Wrq, Wo,
                 s=S, mm_dtype=MM_DTYPE):
    ndt = _np_dtype(mm_dtype)
    w_down = np.ascontiguousarray(
        np.concatenate([Wkv_d, Wq_d], axis=1), dtype=ndt)       # [H, 512]
    rope_cc, rope_ss = _rope_tables(s, ndt)
    ones = np.ones((128, 128), dtype=ndt)
    Wk_u4 = Wk_u.reshape(L, NH, RD)
    Wq_u4 = Wq_u.reshape(L, NH, RD)
    Wrq4 = Wrq.reshape(L, NH, RD)
    Wv_u4 = Wv_u.reshape(L, NH, HD)
    Wrk4 = Wrk.reshape(H, NH, RD)
    Wo4 = Wo.reshape(NH, HD, H)

    def pack_lo_hi(w4, hsel, dim0):
        # [dim0, 4 heads, 64] -> cols [h0_lo..h3_lo, h0_hi..h3_hi]
        wl = w4[:, hsel, 0:RD // 2]                  # [d, 4, 32]
        wh = w4[:, hsel, RD // 2:RD]
        return np.ascontiguousarray(np.concatenate(
            [wl.reshape(dim0, HPC * 32), wh.reshape(dim0, HPC * 32)],
            axis=1), dtype=ndt)                      # [d, 256]

    def pack_qr_pairs(w4, hsel):
        # per pair p: [h(2p)_lo, h(2p+1)_lo, h(2p)_hi, h(2p+1)_hi] (32 each)
        cols = []
        heads = list(range(hsel.start, hsel.stop))
        for p in range(2):
            ha, hb = heads[2 * p], heads[2 * p + 1]
            cols.extend([w4[:, ha, 0:32], w4[:, hb, 0:32],
                         w4[:, ha, 32:64], w4[:, hb, 32:64]])
        return np.ascontiguousarray(
            np.concatenate(cols, axis=1), dtype=ndt)  # [L, 256]

    hsT_b = [np.ascontiguousarray(hidden_states[b, :s].T, dtype=ndt)
             for b in range(B)]                       # [H, s] each

    in_maps = []
    for c in range(8):
        b, g = divmod(c, 4)
        hsel = slice(g * HPC, (g + 1) * HPC)
        in_maps.append({
            "hsT": hsT_b[b],
            "w_down": w_down,
            # k rope: all-lo then all-hi packing (matches k_rT chunks)
            "w_rk": pack_lo_hi(Wrk4, hsel, H),
            "w_qc": np.ascontiguousarray(
                Wq_u4[:, hsel, :].reshape(L, HPC * RD), dtype=ndt),
            "w_qr": pack_qr_pairs(Wrq4, hsel),
            "w_ku": np.ascontiguousarray(
                Wk_u4[:, hsel, :].reshape(L, HPC * RD), dtype=ndt),
            "w_vu": np.ascontiguousarray(
                Wv_u4[:, hsel, :].reshape(L, HPC * HD), dtype=ndt),
            "w_o": np.ascontiguousarray(
                Wo4[hsel].reshape(HPC * HD, H), dtype=ndt),
            "ones_in": ones,
            "rope_cc": rope_cc,
            "rope_ss": rope_ss,
        })
    return in_maps


_NC_CACHE = {}


def kernel(hidden_states, Wkv_d, Wq_d, Wk_u, Wq_u, Wv_u, Wrk, Wrq, Wo):
    from concourse.bass_utils import run_bass_kernel_spmd

    key = (S, MM_DTYPE)
    if key not in _NC_CACHE:
        _NC_CACHE[key] = build_nc(S, MM_DTYPE)
    nc = _NC_CACHE[key]

    in_maps = make_in_maps(
        np.asarray(hidden_states), np.asarray(Wkv_d), np.asarray(Wq_d),
        np.asarray(Wk_u), np.asarray(Wq_u), np.asarray(Wv_u),
        np.asarray(Wrk), np.asarray(Wrq), np.asarray(Wo))

    res = run_bass_kernel_spmd(nc, in_maps, core_ids=list(range(8)))
    parts = [r["out"].astype(np.float32) for r in res.results]
    out = np.empty((B, S, H), dtype=np.float32)
    for b in range(B):
        out[b] = parts[4 * b] + parts[4 * b + 1] + parts[4 * b + 2] + parts[4 * b + 3]
    return out


# revision 27
# speedup vs baseline: 1.3139x; 1.0009x over previous
"""Trainium2 Bass kernel for MultiHeadLatentAttention (MLA), 8-core SPMD.

Sharding: core c = (batch b=c//4, head-group g=c%4 of 4 heads).
Each core computes the full latent down-projections for its batch
(replicated across the 4 cores of that batch), head-sharded
up-projections + RoPE + causal attention, and a partial o-projection
(its heads' rows of Wo). The host sums the 4 partial outputs per batch.

Shapes (fixed): B=2, S=2048, H=2048, L=256, nh=16, hd=128, rd=64.

All matmul operands are bf16 (f32 PSUM accumulation); the host passes
hidden_states pre-transposed ([H, s], features on partitions) so the
device runs no transposes at all.

Device layouts (features on partitions):
  hsT         [128, NKC, s-block] streamed per s-block from DRAM
  kv_dT, q_dT [L=256 -> 2x128, s]
  k_rT        [2x128, s] chunk0 = all heads' rope-lo rows (4x32),
                         chunk1 = all heads' rope-hi rows; rotated in place
  qT_h, kT_h  [128 (64 content + 64 rope), s] per head
  v_all       [128 (s%128), s//128, 4 heads * 128]  (natural v)
  yT_all      [128 (hd), 4 heads, s]

Attention is computed in scores-transposed orientation S^T[k, q] so the
probabilities feed the AV matmul directly (lhsT = v block, rhs = expS).
Softmax skips the max-subtraction (scores are tiny here: |s| < ~2).
Causal structure: k-blocks strictly above the diagonal are skipped, and
diagonal k-blocks compute only the live column range, with a single
[128,128] affine_select for the triangular edge. The denominator is
accumulated per k-block by an all-ones matmul into PSUM (broadcasts it
across partitions); the reciprocal runs on the ACT engine straight from
PSUM and one vector multiply normalizes y.
"""

import os
import sys

sys.path.insert(0, "/opt/trn_rl_repo")

import numpy as np

B = 2
S = 2048
H = 2048
L = 256          # latent dim (2 chunks of 128)
NH = 16          # total heads
HPC = 4          # heads per core
HD = 128         # head dim
RD = 64          # rope / content half-width
ROPE_BASE = 10000.0
SCALE = float(HD) ** -0.5

SB = 512         # s-block for projections / q-blocks in attention
KB = 128         # k-block in attention
NKC = H // 128   # 16 contraction chunks over H
NLC = L // 128   # 2 contraction chunks over L

# Matmul input dtype: "bf16" (fast, ~2e-3 rel err) or "f32r" (~3e-4).
MM_DTYPE = os.environ.get("MLA_MM_DTYPE", "bf16")


def build_nc(s=S, mm_dtype=MM_DTYPE):
    """Build the Bass module for one core. `s` can be shrunk (multiple of 512)
    for simulator testing."""
    from concourse import bacc
    import concourse.bass as bass
    import concourse.mybir as mybir
    import concourse.tile as tile
    from contextlib import ExitStack

    f32 = mybir.dt.float32
    mdt = mybir.dt.bfloat16 if mm_dtype == "bf16" else mybir.dt.float32r

    nsb = s // SB        # s-blocks
    nsc = s // 128       # 128-row s-chunks

    nc = bacc.Bacc(None, target_bir_lowering=False)

    hsT = nc.dram_tensor("hsT", [H, s], mdt, kind="ExternalInput")
    w_down = nc.dram_tensor("w_down", [H, 512], mdt, kind="ExternalInput")
    w_rk = nc.dram_tensor("w_rk", [H, HPC * RD], mdt, kind="ExternalInput")
    w_qc = nc.dram_tensor("w_qc", [L, HPC * RD], mdt, kind="ExternalInput")
    w_qr = nc.dram_tensor("w_qr", [L, HPC * RD], mdt, kind="ExternalInput")
    w_ku = nc.dram_tensor("w_ku", [L, HPC * RD], mdt, kind="ExternalInput")
    w_vu = nc.dram_tensor("w_vu", [L, HPC * HD], mdt, kind="ExternalInput")
    w_o = nc.dram_tensor("w_o", [HPC * HD, H], mdt, kind="ExternalInput")
    ones_in = nc.dram_tensor("ones_in", [128, 128], mdt, kind="ExternalInput")
    # cos/sin halves replicated across all four 32-partition quadrants so any
    # 32-row operand can pair with a table slice at the SAME base partition
    # (walrus: both-SBUF tensor_tensor inputs must share base partition).
    rope_cc = nc.dram_tensor("rope_cc", [128, s], mdt, kind="ExternalInput")
    rope_ss = nc.dram_tensor("rope_ss", [128, s], mdt, kind="ExternalInput")
    out = nc.dram_tensor("out", [s, H], mdt, kind="ExternalOutput")

    Exp = mybir.ActivationFunctionType.Exp
    Ln = mybir.ActivationFunctionType.Ln
    is_ge = mybir.AluOpType.is_ge

    with ExitStack() as top:
        tc = top.enter_context(tile.TileContext(nc))

        # ---- persistent small pools -------------------------------------
        const_pool = top.enter_context(tc.tile_pool(name="const", bufs=1))
        ones128 = const_pool.tile([128, 128], mdt, tag="ones")
        cc_t = const_pool.tile([128, s], mdt, tag="ropec")
        ss_t = const_pool.tile([128, s], mdt, tag="ropes")

        wsmall = top.enter_context(tc.tile_pool(name="wsmall", bufs=1))
        w_qc_t = wsmall.tile([128, NLC, HPC * RD], mdt, tag="wqc")
        w_qr_t = wsmall.tile([128, NLC, HPC * RD], mdt, tag="wqr")
        w_ku_t = wsmall.tile([128, NLC, HPC * RD], mdt, tag="wku")
        w_vu_t = wsmall.tile([128, NLC, HPC * HD], mdt, tag="wvu")

        # ---- latent / rope-k tensors (live through all C1 phases) -------
        lat_pool = top.enter_context(tc.tile_pool(name="lat", bufs=1))
        kv_dT = lat_pool.tile([128, NLC, s], mdt, tag="kvd")   # [L, s]
        q_dT = lat_pool.tile([128, NLC, s], mdt, tag="qd")     # [L, s]
        k_rT = lat_pool.tile([128, NLC, s], mdt, tag="krt")    # rope k rows

        # ============ unified software pipeline over s-blocks ============
        # Per iteration sb: A(sb) latent/rope-k projections (two 3-stream
        # passes so A holds only 3 PSUM banks), k-rope rotation, C1(sb)
        # q/k/v head projections, D(sb-1) o-projection (independent PE
        # filler while C1's DVE chains drain), then C2(qi=sb) causal
        # attention.  PSUM: A 3 + shared C1/score/D ring 3 + py 2 = 8.
        # The softmax denominator is accumulated on DVE (es_acc) and
        # reduced by a single all-ones matmul per (head, q-block).
        hsT_r = hsT.rearrange("(ko p) m -> p ko m", p=128)

        vp = top.enter_context(tc.tile_pool(name="vp", bufs=1))
        yp = top.enter_context(tc.tile_pool(name="yp", bufs=1))
        yT_all = yp.tile([128, HPC, s], mdt, tag="yT")

        hstp = top.enter_context(tc.tile_pool(name="hst", bufs=2))
        wdp = top.enter_context(tc.tile_pool(name="wdown", bufs=1))
        wrkp = top.enter_context(tc.tile_pool(name="wrk", bufs=1))
        qkp = top.enter_context(tc.tile_pool(name="qkp", bufs=2))
        esp = top.enter_context(tc.tile_pool(name="esp", bufs=4))
        accp = top.enter_context(tc.tile_pool(name="accp", bufs=4))
        recp = top.enter_context(tc.tile_pool(name="recp", bufs=2))
        rtmp = top.enter_context(tc.tile_pool(name="rtmp", bufs=2))
        rkr = top.enter_context(tc.tile_pool(name="rkr", bufs=2))
        wop = top.enter_context(tc.tile_pool(name="wop", bufs=4))
        outp = top.enter_context(tc.tile_pool(name="outp", bufs=4))

        psA = top.enter_context(tc.tile_pool(name="psA", bufs=3, space="PSUM"))
        psC = top.enter_context(tc.tile_pool(name="psC", bufs=3, space="PSUM"))
        ps_y = top.enter_context(tc.tile_pool(name="ps_y", bufs=2, space="PSUM"))

        nqb = s // SB
        dpq = SB // KB                      # diagonal blocks per qi

        # weight / table DMAs, chunked so the first matmuls start early
        w_down_t = wdp.tile([128, NKC, 512], mdt, tag="wd")
        w_rk_t = wrkp.tile([128, NKC, HPC * RD], mdt, tag="wr")
        wd_r = w_down.rearrange("(ko p) m -> p ko m", p=128)
        wr_r = w_rk.rearrange("(ko p) m -> p ko m", p=128)
        for kc in range(NKC):
            nc.sync.dma_start(w_down_t[:, kc, :], wd_r[:, kc, :])
            nc.sync.dma_start(w_rk_t[:, kc, :], wr_r[:, kc, :])
        nc.sync.dma_start(ones128[:], ones_in[:])
        nc.sync.dma_start(cc_t[:], rope_cc[:])
        nc.sync.dma_start(ss_t[:], rope_ss[:])
        nc.sync.dma_start(
            w_qc_t[:], w_qc.rearrange("(ko p) m -> p ko m", p=128))
        nc.sync.dma_start(
            w_qr_t[:], w_qr.rearrange("(ko p) m -> p ko m", p=128))
        nc.sync.dma_start(
            w_ku_t[:], w_ku.rearrange("(ko p) m -> p ko m", p=128))
        nc.sync.dma_start(
            w_vu_t[:], w_vu.rearrange("(ko p) m -> p ko m", p=128))
        hs_tiles = {}

        def load_hs(sb, nchunk=4):
            if sb >= nsb:
                return
            t = hstp.tile([128, NKC, SB], mdt, tag="hsT", name=f"hsT_{sb}")
            w = NKC // nchunk
            for g in range(nchunk):
                nc.gpsimd.dma_start(
                    t[:, w * g:w * (g + 1), :],
                    hsT_r[:, w * g:w * (g + 1), sb * SB:(sb + 1) * SB])
            hs_tiles[sb] = t

        qT = {h: qkp.tile([128, s], mdt, tag="qT", bufs=4,
                          name=f"qT_{h}") for h in range(2 * HPC // 2)}
        kT = {h: qkp.tile([128, s], mdt, tag="kT", bufs=4,
                          name=f"kT_{h}") for h in range(2 * HPC // 2)}
        v_pair = {hp: vp.tile([128, nsc, 2 * HD], mdt, tag="vpair",
                              bufs=2, name=f"v_pair_{hp}")
                  for hp in range(2)}

        def emit_a(sb):
            # latent + rope-k projections for s-block sb, two 3-stream
            # passes over the same hsT tile (3 PSUM banks)
            sbs = slice(sb * SB, (sb + 1) * SB)
            hsTt = hs_tiles.pop(sb)
            passes = (
                (("kv0", 0, kv_dT, 0), ("kv1", 128, kv_dT, 1),
                 ("q0", 256, q_dT, 0)),
                (("q1", 384, q_dT, 1), ("kr0", 512, k_rT, 0),
                 ("kr1", 640, k_rT, 1)),
            )
            for pi, streams in enumerate(passes):
                if pi == 1:
                    # next s-block's hsT load starts only now, so this
                    # block's stream gets the full HBM bandwidth early
                    load_hs(sb + 1)
                pb = {}
                for n, off, _, _ in streams:
                    pb[n] = psA.tile([128, SB], f32, tag="pa",
                                     name=f"pa_{n}_{sb}")
                for kc in range(NKC):
                    rhs = hsTt[:, kc, :]
                    st = kc == 0
                    sp = kc == NKC - 1
                    for n, off, _, _ in streams:
                        if off < 512:
                            w = w_down_t[:, kc, off:off + 128]
                        else:
                            w = w_rk_t[:, kc, off - 512:off - 384]
                        nc.tensor.matmul(pb[n][:], w, rhs, start=st, stop=sp)
                for i, (n, off, dst, lc) in enumerate(streams):
                    if i % 2 == 0:
                        nc.scalar.copy(dst[:, lc, sbs], pb[n][:])
                    else:
                        nc.vector.tensor_copy(dst[:, lc, sbs], pb[n][:])

        def emit_rot(sb):
            # rotate k_rT in place (RoPE on all 4 heads at once):
            # chunk0 rows = all heads' lo dims, chunk1 = hi dims:
            #   lo' = lo*cos - hi*sin ;  hi' = hi*cos + lo*sin
            sbs = slice(sb * SB, (sb + 1) * SB)
            r1 = rkr.tile([128, SB], mdt, tag="r1")
            r2 = rkr.tile([128, SB], mdt, tag="r2")
            r3 = rkr.tile([128, SB], mdt, tag="r3")
            r4 = rkr.tile([128, SB], mdt, tag="r4")
            lo = k_rT[:, 0, sbs]
            hi = k_rT[:, 1, sbs]
            nc.vector.tensor_mul(r1[:], lo, cc_t[:, sbs])
            nc.vector.tensor_mul(r2[:], hi, ss_t[:, sbs])
            nc.vector.tensor_mul(r3[:], hi, cc_t[:, sbs])
            nc.vector.tensor_mul(r4[:], lo, ss_t[:, sbs])
            nc.vector.tensor_sub(k_rT[:, 0, sbs], r1[:], r2[:])
            nc.vector.tensor_add(k_rT[:, 1, sbs], r3[:], r4[:])

        def emit_c1_b(hp, sb):
            h0, h1 = 2 * hp, 2 * hp + 1
            sbs = slice(sb * SB, (sb + 1) * SB)
            # q rope FIRST: its DVE chain is the longest pole to the
            # first score matmul of this q-block.
            # psum rows = [h0_lo, h1_lo, h0_hi, h1_hi] (32 each)
            pr = psC.tile([128, SB], f32, tag="blk",
                          name=f"pr_{hp}_{sb}")
            for lc in range(NLC):
                nc.tensor.matmul(
                    pr[:],
                    w_qr_t[:, lc, hp * 128:(hp + 1) * 128],
                    q_dT[:, lc, sbs],
                    start=(lc == 0), stop=(lc == NLC - 1))
            t1 = rtmp.tile([64, SB], mdt, tag="t1")
            t2 = rtmp.tile([64, SB], mdt, tag="t2")
            t3 = rtmp.tile([64, SB], mdt, tag="t3")
            t4 = rtmp.tile([64, SB], mdt, tag="t4")
            nc.vector.tensor_mul(t1[:], pr[0:64, :], cc_t[0:64, sbs])
            nc.vector.tensor_mul(t2[:], pr[64:128, :], ss_t[64:128, sbs])
            nc.vector.tensor_mul(t3[:], pr[64:128, :], cc_t[64:128, sbs])
            nc.vector.tensor_mul(t4[:], pr[0:64, :], ss_t[0:64, sbs])
            # write rotated rows straight into each head's qT
            nc.vector.tensor_sub(qT[h0][64:96, sbs],
                                 t1[0:32, :], t2[0:32, :])
            nc.vector.tensor_sub(qT[h1][64:96, sbs],
                                 t1[32:64, :], t2[32:64, :])
            nc.vector.tensor_add(qT[h0][96:128, sbs],
                                 t3[0:32, :], t4[0:32, :])
            nc.vector.tensor_add(qT[h1][96:128, sbs],
                                 t3[32:64, :], t4[32:64, :])

            # k content for both heads in one [128, SB] psum.  All
            # evictions on DVE so ACT stays nearly exp-only (the exp
            # feeds the AV matmul critical path).
            pk = psC.tile([128, SB], f32, tag="blk",
                          name=f"pk_{hp}_{sb}")
            for lc in range(NLC):
                nc.tensor.matmul(
                    pk[:],
                    w_ku_t[:, lc, hp * 128:(hp + 1) * 128],
                    kv_dT[:, lc, sbs],
                    start=(lc == 0), stop=(lc == NLC - 1))
            nc.scalar.copy(kT[h0][0:64, sbs], pk[0:64, :])
            nc.scalar.copy(kT[h1][0:32, sbs], pk[64:96, :])
            nc.scalar.copy(kT[h1][32:64, sbs], pk[96:128, :])
            # k rope: copy pre-rotated k_rT rows on DVE
            for h in (h0, h1):
                rb = slice(32 * h, 32 * h + 32)
                nc.vector.tensor_copy(kT[h][64:96, sbs],
                                      k_rT[rb, 0, sbs])
                nc.vector.tensor_copy(kT[h][96:128, sbs],
                                      k_rT[rb, 1, sbs])

            # q content for both heads in one psum
            pc = psC.tile([128, SB], f32, tag="blk",
                          name=f"pc_{hp}_{sb}")
            for lc in range(NLC):
                nc.tensor.matmul(
                    pc[:],
                    w_qc_t[:, lc, hp * 128:(hp + 1) * 128],
                    q_dT[:, lc, sbs],
                    start=(lc == 0), stop=(lc == NLC - 1))
            nc.scalar.copy(qT[h0][0:64, sbs], pc[0:64, :])
            nc.scalar.copy(qT[h1][0:32, sbs], pc[64:96, :])
            nc.scalar.copy(qT[h1][32:64, sbs], pc[96:128, :])

            # B: v for this s-block (natural layout); pv feeds the
            # imminent AV matmuls, so keep its eviction fast on ACT
            for sc in range(4 * sb, 4 * sb + 4):
                pv = psC.tile([128, SB], f32, tag="blk",
                              name=f"pv_{hp}_{sc}")
                for lc in range(NLC):
                    nc.tensor.matmul(
                        pv[:, 0:2 * HD],
                        kv_dT[:, lc, sc * 128:(sc + 1) * 128],
                        w_vu_t[:, lc, hp * 2 * HD:(hp + 1) * 2 * HD],
                        start=(lc == 0), stop=(lc == NLC - 1))
                nc.scalar.copy(v_pair[hp][:, sc, :], pv[:, 0:2 * HD])

        def emit_c2(hp, qi):
            h0, h1 = 2 * hp, 2 * hp + 1
            nkj = (qi + 1) * dpq
            qs = slice(qi * SB, (qi + 1) * SB)
            py = {h: ps_y.tile([128, SB], f32, tag="py",
                               name=f"py_{h}_{qi}")
                  for h in (h0, h1)}
            acc = {h: accp.tile([128, SB], mdt, tag="acc",
                                name=f"acc_{h}_{qi}")
                   for h in (h0, h1)}

            def score(h, kj, c0):
                ps = psC.tile([128, SB], f32, tag="blk",
                              name=f"ps_{h}_{qi}_{kj}")
                nc.tensor.matmul(
                    ps[:, c0:SB], kT[h][:, kj * KB:(kj + 1) * KB],
                    qT[h][:, qi * SB + c0:(qi + 1) * SB],
                    start=True, stop=True)
                return ps

            def finish(h, kj, c0, ps):
                es = esp.tile([128, SB], mdt, tag="es",
                              name=f"es_{h}_{qi}_{kj}")
                nc.scalar.activation(es[:, c0:SB], ps[:, c0:SB],
                                     Exp, scale=SCALE)
                if kj >= qi * dpq:          # diagonal block
                    nc.gpsimd.affine_select(
                        out=es[:, c0:c0 + KB], in_=es[:, c0:c0 + KB],
                        compare_op=is_ge, fill=0.0,
                        base=0, pattern=[[1, KB]],
                        channel_multiplier=-1)
                nc.tensor.matmul(
                    py[h][:, c0:SB],
                    v_pair[hp][:, kj, (h - h0) * HD:(h - h0 + 1) * HD],
                    es[:, c0:SB], start=(kj == 0), stop=(kj == nkj - 1))
                # denominator accumulation on DVE (kj=0 is always full
                # width, so the copy initializes the whole tile)
                if kj == 0:
                    nc.vector.tensor_copy(acc[h][:], es[:])
                else:
                    nc.vector.tensor_add(acc[h][:, c0:SB],
                                         acc[h][:, c0:SB], es[:, c0:SB])

            pend = []
            for kj in range(nkj):
                c0 = max(0, (kj - qi * dpq) * KB)
                for h in (h0, h1):
                    pend.append((h, kj, c0, score(h, kj, c0)))
                    if len(pend) > 3:
                        finish(*pend.pop(0))
            for it in pend:
                finish(*it)

            for h in (h0, h1):
                # partition-reduce the denominator with one all-ones
                # matmul (broadcasts den across partitions), then 1/den
                # via the fast Newton-Raphson DVE op (~18 correct bits;
                # den is in [1, ~2e3] so no edge cases) and one DVE
                # multiply (PSUM x SBUF)
                pden = psC.tile([128, SB], f32, tag="blk",
                                name=f"pden_{h}_{qi}")
                nc.tensor.matmul(pden[:], ones128[:], acc[h][:],
                                 start=True, stop=True)
                rec = recp.tile([128, SB], f32, tag="rec",
                                name=f"rec_{h}_{qi}")
                nc.vector.reciprocal_approx_fast(
                    out=rec[:], in_=pden[:])
                nc.vector.tensor_mul(yT_all[:, h, qs],
                                     py[h][:], rec[:])

        def emit_d(qi):
            # o-projection for q-block qi (all 4 heads' yT ready)
            for sc in range(4 * qi, 4 * qi + 4):
                for ncol in range(H // 512):
                    po = psC.tile([128, SB], f32, tag="blk",
                                  name=f"po_{sc}_{ncol}")
                    for hh in range(HPC):
                        nc.tensor.matmul(
                            po[:], yT_all[:, hh, sc * 128:(sc + 1) * 128],
                            wo_tiles[ncol][:, hh, :],
                            start=(hh == 0), stop=(hh == HPC - 1))
                    ot = outp.tile([128, 512], mdt, tag="ot")
                    if ncol % 2 == 0:
                        nc.scalar.copy(ot[:], po[:])
                    else:
                        nc.vector.tensor_copy(ot[:], po[:])
                    nc.sync.dma_start(
                        out[sc * 128:(sc + 1) * 128,
                            ncol * 512:(ncol + 1) * 512], ot[:])

        load_hs(0, nchunk=8)
        wo_tiles = []
        for sb in range(nsb):
            emit_a(sb)
            emit_rot(sb)
            for hp in range(2):
                emit_c1_b(hp, sb)
            if sb > 0:
                emit_d(sb - 1)
            for hp in range(2):
                emit_c2(hp, sb)
            if sb == 0:
                # o-proj weights late on the hsT queue: clear of the
                # startup window, well before first use (~110us)
                for ncol in range(H // 512):
                    wo_t = wop.tile([128, HPC, 512], mdt, tag="wo",
                                    name=f"wo_{ncol}")
                    nc.gpsimd.dma_start(
                        wo_t[:],
                        w_o[:, ncol * 512:(ncol + 1) * 512].rearrange(
                            "(ho p) m -> p ho m", p=128))
                    wo_tiles.append(wo_t)
        emit_d(nsb - 1)

    nc.compile()
    return nc


# ======================= host-side preparation ==========================

def _np_dtype(mm_dtype):
    if mm_dtype == "bf16":
        import ml_dtypes

        return ml_dtypes.bfloat16
    return np.float32


def _rope_tables(s, ndt):
    inv_freq = 1.0 / (ROPE_BASE ** (np.arange(0, RD, 2, dtype=np.float64) / RD))
    t = np.arange(s, dtype=np.float64)
    freqs = np.outer(t, inv_freq)                    # [s, 32]
    cc = np.tile(np.cos(freqs).T, (4, 1)).astype(ndt)   # [128, s]
    ss = np.tile(np.sin(freqs).T, (4, 1)).astype(ndt)
    return np.ascontiguousarray(cc), np.ascontiguousarray(ss)


def make_in_maps(hidden_states, Wkv_d, Wq_d, Wk_u, Wq_u, Wv_u, Wrk, Wrq, Wo,
                 s=S, mm_dtype=MM_DTYPE):
    ndt = _np_dtype(mm_dtype)
    w_down = np.ascontiguousarray(
        np.concatenate([Wkv_d, Wq_d], axis=1), dtype=ndt)       # [H, 512]
    rope_cc, rope_ss = _rope_tables(s, ndt)
    ones = np.ones((128, 128), dtype=ndt)
    Wk_u4 = Wk_u.reshape(L, NH, RD)
    Wq_u4 = Wq_u.reshape(L, NH, RD)
    Wrq4 = Wrq.reshape(L, NH, RD)
    Wv_u4 = Wv_u.reshape(L, NH, HD)
    Wrk4 = Wrk.reshape(H, NH, RD)
    Wo4 = Wo.reshape(NH, HD, H)

    def pack_lo_hi(w4, hsel, dim0):
        # [dim0, 4 heads, 64] -> cols [h0_lo..h3_lo, h0_hi..h3_hi]
        wl = w4[:, hsel, 0:RD // 2]                  # [d, 4, 32]
        wh = w4[:, hsel, RD // 2:RD]
        return np.ascontiguousarray(np.concatenate(
            [wl.reshape(dim0, HPC * 32), wh.reshape(dim0, HPC * 32)],
            axis=1), dtype=ndt)                      # [d, 256]

    def pack_qr_pairs(w4, hsel):
        # per pair p: [h(2p)_lo, h(2p+1)_lo, h(2p)_hi, h(2p+1)_hi] (32 each)
        cols = []
        heads = list(range(hsel.start, hsel.stop))
        for p in range(2):
            ha, hb = heads[2 * p], heads[2 * p + 1]
            cols.extend([w4[:, ha, 0:32], w4[:, hb, 0:32],
                         w4[:, ha, 32:64], w4[:, hb, 32:64]])
        return np.ascontiguousarray(
            np.concatenate(cols, axis=1), dtype=ndt)  # [L, 256]

    hsT_b = [np.ascontiguousarray(hidden_states[b, :s].T, dtype=ndt)
             for b in range(B)]                       # [H, s] each

    in_maps = []
    for c in range(8):
        b, g = divmod(c, 4)
        hsel = slice(g * HPC, (g + 1) * HPC)
        in_maps.append({
            "hsT": hsT_b[b],
            "w_down": w_down,
            # k rope: all-lo then all-hi packing (matches k_rT chunks)
            "w_rk": pack_lo_hi(Wrk4, hsel, H),
            "w_qc": np.ascontiguousarray(
                Wq_u4[:, hsel, :].reshape(L, HPC * RD), dtype=ndt),
            "w_qr": pack_qr_pairs(Wrq4, hsel),
            "w_ku": np.ascontiguousarray(
                Wk_u4[:, hsel, :].reshape(L, HPC * RD), dtype=ndt),
            "w_vu": np.ascontiguousarray(
                Wv_u4[:, hsel, :].reshape(L, HPC * HD), dtype=ndt),
            "w_o": np.ascontiguousarray(
                Wo4[hsel].reshape(HPC * HD, H), dtype=ndt),
            "ones_in": ones,
            "rope_cc": rope_cc,
            "rope_ss": rope_ss,
        })
    return in_maps


_NC_CACHE = {}


def kernel(hidden_states, Wkv_d, Wq_d, Wk_u, Wq_u, Wv_u, Wrk, Wrq, Wo):
    from concourse.bass_utils import run_bass_kernel_spmd

    key = (S, MM_DTYPE)
    if key not in _NC_CACHE:
        _NC_CACHE[key] = build_nc(S, MM_DTYPE)
    nc = _NC_CACHE[key]

    in_maps = make_in_maps(
        np.asarray(hidden_states), np.asarray(Wkv_d), np.asarray(Wq_d),
        np.asarray(Wk_u), np.asarray(Wq_u), np.asarray(Wv_u),
        np.asarray(Wrk), np.asarray(Wrq), np.asarray(Wo))

    res = run_bass_kernel_spmd(nc, in_maps, core_ids=list(range(8)))
    parts = [r["out"].astype(np.float32) for r in res.results]
    out = np.empty((B, S, H), dtype=np.float32)
    for b in range(B):
        out[b] = parts[4 * b] + parts[4 * b + 1] + parts[4 * b + 2] + parts[4 * b + 3]
    return out
